# revision 19
# baseline (speedup 1.0000x reference)
"""Trainium2 Bass kernel for nn_BasicLayer (sparse cluster attention, 2 layers).

v2 rewrite of the staged baseline. Same host-side strategy (scanline gather,
8 cores x 8192 tokens, folded weights, token-major fp32 residual, bf16 matmul
operands) with an on-device restructure aimed at engine balance and PE
density:

- All layout flips (LN token-major -> feature-major, P -> P^T) go through the
  DMA xbar transpose engine instead of PE transpose + PSUM copy.
- LN: one batched bn_stats pair, rsqrt via fast-inverse-sqrt bit trick +
  1 Newton step (DVE only, no sqrt table), normalize via dual-scalar
  tensor_scalar producing bf16 directly.
- Scores matmuls read per-head q/k slices in place via tile_position row
  packing (no per-head copies).
- Softmax: exp -> scratch E; P = E * (1/rowsum) only on the diagonal
  64x64 blocks into persistent zeroed P buffers (gpsimd); P^T via DMA
  transpose feeds the O matmuls.
- Supertiles processed in groups of 4 with phase-major ordering per layer so
  the scalar engine's activation-table switches (Exp <-> Gelu) amortize
  across the group.
"""

import os
import numpy as np
import ml_dtypes

# ---- problem constants (hardcoded per contract) ----
B, N, D = 4, 16384, 192
DP = 256
HEADS, DH, CLM = 6, 32, 64
GRID_W = 128
DEPTH = 2
NCORES = 8
T = (B * N) // NCORES                # 8192 tokens per core
SUB = 128
NSUB = 4
TILE = SUB * NSUB                    # 512-token supertile
NTILES = T // TILE                   # 16
GROUP = 8                            # supertiles per phase group
DFF = 768

_COMPILED = {}


def _scanline_order(pos, w):
    ix = np.floor(pos[..., 0]).astype(np.int64)
    iy = np.floor(pos[..., 1]).astype(np.int64)
    key = iy * w + np.where(iy % 2 == 1, w - 1 - ix, ix)
    return np.argsort(key, axis=1, kind="stable")


def _fold_weights(inputs):
    """Fold LN affine + biases into matmul weights (same layout as v1)."""
    bf16 = ml_dtypes.bfloat16
    scale = DH ** -0.5
    layers = []
    for i in range(DEPTH):
        g1 = np.asarray(inputs["ln1_g"][i], np.float64)
        b1 = np.asarray(inputs["ln1_b"][i], np.float64)
        Wqkv = np.asarray(inputs["w_qkv"][i], np.float64)
        bqkv = np.asarray(inputs["b_qkv"][i], np.float64)
        w_eff = g1[:, None] * Wqkv
        b_eff = b1 @ Wqkv + bqkv
        wq = w_eff[:, 0:D] * scale
        bq = b_eff[0:D] * scale
        wk = w_eff[:, D:2 * D]
        bk = b_eff[D:2 * D]
        wv = w_eff[:, 2 * D:3 * D]
        bv = b_eff[2 * D:3 * D]
        wqk = np.concatenate(
            [wq[:, :128], wk[:, :128], wq[:, 128:], wk[:, 128:]], axis=1)
        pad64 = np.zeros(64)
        bqk = np.stack(
            [bq[:128], bk[:128],
             np.concatenate([bq[128:], pad64]),
             np.concatenate([bk[128:], pad64])], axis=1)
        wp = np.asarray(inputs["w_proj"][i], np.float64)
        bp = np.asarray(inputs["b_proj"][i], np.float64)
        g2 = np.asarray(inputs["ln2_g"][i], np.float64)
        b2 = np.asarray(inputs["ln2_b"][i], np.float64)
        W1 = np.asarray(inputs["w_fc1"][i], np.float64)
        w1_eff = g2[:, None] * W1
        b1_eff = b2 @ W1 + np.asarray(inputs["b_fc1"][i], np.float64)
        W2 = np.asarray(inputs["w_fc2"][i], np.float64)
        bfc2 = np.asarray(inputs["b_fc2"][i], np.float64)
        bv_t = np.stack(
            [bv[:128], np.concatenate([bv[128:], np.zeros(64)])], axis=1)
        layers.append({
            f"wqk{i}": wqk.astype(bf16),
            f"bqk{i}": bqk.astype(np.float32),
            f"wv{i}": wv.astype(bf16),
            f"bv{i}": bv_t.astype(np.float32),
            f"wp{i}": wp.astype(bf16),
            f"bp{i}": np.tile(bp.astype(np.float32), (128, 1)),
            f"w1{i}": w1_eff.astype(bf16),
            f"b1{i}": b1_eff.reshape(6, 128).T.copy().astype(np.float32),
            f"w2{i}": W2.astype(bf16),
            f"b2{i}": np.tile(bfc2.astype(np.float32), (128, 1)),
        })
    return layers


def _build_nc(biases_zero=True, ntiles=NTILES):
    key = ("nc", biases_zero, ntiles)
    if key in _COMPILED:
        return _COMPILED[key]

    from contextlib import ExitStack
    import concourse.bass as bass
    import concourse.tile as tile
    from concourse import bacc, mybir
    from concourse.bass import ts, ds

    f32 = mybir.dt.float32
    bf16 = mybir.dt.bfloat16
    i32 = mybir.dt.int32
    AF = mybir.ActivationFunctionType
    OP = mybir.AluOpType

    tok_total = ntiles * TILE

    nc = bacc.Bacc("TRN2", target_bir_lowering=False, debug=False,
                   enable_asserts=False, num_devices=NCORES)

    x_d = nc.dram_tensor("x", [ntiles, 128, NSUB * DP], f32,
                         kind="ExternalInput").ap()
    y_d = nc.dram_tensor("y", [ntiles, 128, NSUB * D], f32,
                         kind="ExternalOutput").ap()
    wd = []
    for i in range(DEPTH):
        wd.append({
            "wqk": nc.dram_tensor(f"wqk{i}", [D, 384], bf16, kind="ExternalInput").ap(),
            "bqk": nc.dram_tensor(f"bqk{i}", [128, 4], f32, kind="ExternalInput").ap(),
            "wv": nc.dram_tensor(f"wv{i}", [D, D], bf16, kind="ExternalInput").ap(),
            "bv": nc.dram_tensor(f"bv{i}", [128, 2], f32, kind="ExternalInput").ap(),
            "wp": nc.dram_tensor(f"wp{i}", [D, D], bf16, kind="ExternalInput").ap(),
            "bp": nc.dram_tensor(f"bp{i}", [128, D], f32, kind="ExternalInput").ap(),
            "w1": nc.dram_tensor(f"w1{i}", [D, DFF], bf16, kind="ExternalInput").ap(),
            "b1": nc.dram_tensor(f"b1{i}", [128, 6], f32, kind="ExternalInput").ap(),
            "w2": nc.dram_tensor(f"w2{i}", [DFF, D], bf16, kind="ExternalInput").ap(),
            "b2": nc.dram_tensor(f"b2{i}", [128, D], f32, kind="ExternalInput").ap(),
        })

    with tile.TileContext(nc) as tc, ExitStack() as ctx:
        consts = ctx.enter_context(tc.tile_pool(name="consts", bufs=1))
        xpool = ctx.enter_context(tc.tile_pool(name="xpool", bufs=10))
        lnpool = ctx.enter_context(tc.tile_pool(name="lnpool", bufs=6))
        fmpool = ctx.enter_context(tc.tile_pool(name="fmpool", bufs=10))
        qkpool = ctx.enter_context(tc.tile_pool(name="qkpool", bufs=8))
        apool = ctx.enter_context(tc.tile_pool(name="apool", bufs=3))
        pkpool = ctx.enter_context(tc.tile_pool(name="pkpool", bufs=7))
        ofpool = ctx.enter_context(tc.tile_pool(name="ofpool", bufs=8))
        hpool = ctx.enter_context(tc.tile_pool(name="hpool", bufs=2))
        stpool = ctx.enter_context(tc.tile_pool(name="stpool", bufs=9))
        ppsc = ctx.enter_context(tc.tile_pool(name="ppsc", bufs=1, space="PSUM"))
        ppm = ctx.enter_context(tc.tile_pool(name="ppm", bufs=4, space="PSUM"))

        # persistent softmax buffers: off-diagonal blocks stay 0 forever
        NPBUF = 4
        p_bufs = []
        for pb_i in range(NPBUF):
            pb = consts.tile([128, 2, HEADS, 128], bf16, name=f"pbuf{pb_i}")
            nc.vector.memset(pb, 0.0)
            p_bufs.append(pb)

        # --- load weights into SBUF once ---
        W = []
        for i in range(DEPTH):
            d = wd[i]
            sb = {}
            sb["wqk0"] = consts.tile([128, 384], bf16, name=f"wqk0{i}")
            sb["wqk1"] = consts.tile([64, 384], bf16, name=f"wqk1{i}")
            nc.scalar.dma_start(out=sb["wqk0"], in_=d["wqk"][0:128])
            nc.scalar.dma_start(out=sb["wqk1"], in_=d["wqk"][128:192])
            sb["wv0"] = consts.tile([128, D], bf16, name=f"wv0{i}")
            sb["wv1"] = consts.tile([64, D], bf16, name=f"wv1{i}")
            nc.scalar.dma_start(out=sb["wv0"], in_=d["wv"][0:128])
            nc.scalar.dma_start(out=sb["wv1"], in_=d["wv"][128:192])
            sb["wp0"] = consts.tile([128, D], bf16, name=f"wp0{i}")
            sb["wp1"] = consts.tile([64, D], bf16, name=f"wp1{i}")
            nc.scalar.dma_start(out=sb["wp0"], in_=d["wp"][0:128])
            nc.scalar.dma_start(out=sb["wp1"], in_=d["wp"][128:192])
            sb["w10"] = consts.tile([128, DFF], bf16, name=f"w10{i}")
            sb["w11"] = consts.tile([64, DFF], bf16, name=f"w11{i}")
            nc.scalar.dma_start(out=sb["w10"], in_=d["w1"][0:128])
            nc.scalar.dma_start(out=sb["w11"], in_=d["w1"][128:192])
            sb["w2m"] = consts.tile([128, 6, D], bf16, name=f"w2m{i}")
            nc.scalar.dma_start(
                out=sb["w2m"],
                in_=d["w2"].rearrange("(m p) n -> p m n", p=128))
            for nm in ("bqk", "bv", "b1", "bp", "b2"):
                shp = {"bqk": [128, 4], "bv": [128, 2], "b1": [128, 6],
                       "bp": [128, D], "b2": [128, D]}[nm]
                sb[nm] = consts.tile(shp, f32, name=f"{nm}{i}")
                nc.scalar.dma_start(out=sb[nm], in_=d[nm])
            W.append(sb)

        pair_ctr = [0]
        MAGIC = 0x5F3759DF
        # CoreSim lacks Gelu_apprx_tanh; substitute Tanh for sim-only runs.
        GELU_FUNC = (AF.Tanh if os.environ.get("K_SIM_GELU_TANH") == "1"
                     else AF.Gelu_apprx_tanh)

        def layernorm_fm(x_t, tag):
            """LN on token-major x_t -> feature-major bf16 via DMA transpose.
            Returns fmA [128,4,128] (feats 0:128, cols=tokens) and fmA2
            (feats 128:256; partitions 64:128 are pad)."""
            mv = stpool.tile([128, 4, 6], f32, tag="mv", name="mv")
            mv2 = stpool.tile([128, 4, 2], f32, tag="mv2", name="mv2")
            for s in range(NSUB):
                nc.vector.bn_stats(mv[:, s], x_t[:, s, 0:D])
                nc.vector.bn_aggr(mv2[:, s], mv[:, s])
            var = mv2[:, :, 1]                       # [128, 4] stride 2
            t_i = stpool.tile([128, 4], i32, tag="ti", name="t_i")
            y0 = stpool.tile([128, 4], f32, tag="y0", name="y0")
            zz = stpool.tile([128, 4], f32, tag="zz", name="zz")
            r4 = stpool.tile([128, 4], f32, tag="r4", name="r4")
            nc.vector.tensor_scalar(
                out=t_i, in0=var.bitcast(i32), scalar1=1, scalar2=None,
                op0=OP.logical_shift_right)
            nc.vector.tensor_scalar(
                out=y0.bitcast(i32), in0=t_i, scalar1=MAGIC, scalar2=-1,
                op0=OP.subtract, op1=OP.mult)
            nc.vector.scalar_tensor_tensor(
                out=zz, in0=var, scalar=1e-5, in1=y0,
                op0=OP.add, op1=OP.mult)              # (var+eps)*y0
            nc.vector.tensor_tensor(out=zz, in0=zz, in1=y0, op=OP.mult)
            nc.vector.tensor_scalar(
                out=zz, in0=zz, scalar1=-0.5, scalar2=1.5,
                op0=OP.mult, op1=OP.add)              # 1.5 - 0.5 v y0^2
            nc.vector.tensor_tensor(out=r4, in0=zz, in1=y0, op=OP.mult)

            xn = lnpool.tile([128, 2, NSUB, 128], bf16, tag="xn",
                             name=f"xn{tag}")
            for s in range(NSUB):
                nc.vector.tensor_scalar(
                    out=xn[:, :, s], in0=x_t[:, s].rearrange("p (c f) -> p c f", c=2),
                    scalar1=mv2[:, s, 0:1], scalar2=r4[:, s:s + 1],
                    op0=OP.subtract, op1=OP.mult)
            fm2 = fmpool.tile([128, 2, NSUB, 128], bf16, tag="fm",
                              name=f"fm{tag}")
            nc.sync.dma_start_transpose(out=fm2, in_=xn)
            return fm2[:, 0], fm2[:, 1]

        def phase_a(sb, fmA, fmA2):
            """qkv + v from feature-major LN output. Returns (qkA, qkB, v_tm)."""
            fmAf = fmA.rearrange("p a b -> p (a b)")
            fmA2f = fmA2.rearrange("p a b -> p (a b)")
            # qkv: m-chunks 0,1 are 128-wide (heads 0-3 q|k), 2,3 are 64-wide
            qkA = qkpool.tile([128, 2, TILE], bf16, tag="qkA", name="qkA")
            qkB = qkpool.tile([64, 2, TILE], bf16, tag="qkB", name="qkB")
            psq = []
            for m in range(2):
                ps = ppm.tile([128, TILE], f32, tag="med", name=f"psqA{m}")
                nc.tensor.matmul(ps, sb["wqk0"][:, ts(m, 128)], fmAf,
                                 start=True, stop=False)
                nc.tensor.matmul(ps, sb["wqk1"][:, ts(m, 128)],
                                 fmA2f[0:64], start=False, stop=True)
                psq.append(ps)
            for m in range(2):
                ps = ppm.tile([64, TILE], f32, tag="med", name=f"psqB{m}")
                nc.tensor.matmul(ps, sb["wqk0"][:, ds(256 + m * 64, 64)],
                                 fmAf, start=True, stop=False)
                nc.tensor.matmul(ps, sb["wqk1"][:, ds(256 + m * 64, 64)],
                                 fmA2f[0:64], start=False, stop=True)
                psq.append(ps)
            for m in range(2):
                if biases_zero:
                    nc.scalar.activation(qkA[:, m], psq[m], AF.Copy)
                    nc.vector.tensor_copy(qkB[:, m], psq[2 + m])
                else:
                    nc.scalar.activation(qkA[:, m], psq[m], AF.Identity,
                                         bias=sb["bqk"][:, m:m + 1])
                    nc.scalar.activation(qkB[:, m], psq[2 + m], AF.Identity,
                                         bias=sb["bqk"][0:64, 2 + m:3 + m])
            # v (token-major out), pairs of subs per PSUM tile
            v_tm = qkpool.tile([128, NSUB, D], bf16, tag="vtm", name="v_tm")
            for sp in range(2):
                psv = ppm.tile([128, 2, 256], f32, tag="med", name="psv")
                for j in range(2):
                    s = sp * 2 + j
                    nc.tensor.matmul(psv[:, j, 0:D], fmA[:, s], sb["wv0"],
                                     start=True, stop=False)
                    nc.tensor.matmul(psv[:, j, 0:D], fmA2[0:64, s], sb["wv1"],
                                     start=False, stop=True)
                if biases_zero:
                    nc.scalar.activation(v_tm[:, ds(sp * 2, 2)],
                                         psv[:, :, 0:D], AF.Copy)
                else:
                    for j in range(2):
                        nc.scalar.activation(
                            v_tm[:, sp * 2 + j], psv[:, j, 0:D], AF.Identity,
                            bias=sb["bv"][:, 0:1])
            return qkA, qkB, v_tm

        def phase_b_soft(sb, qkA, qkB, sp):
            """scores + softmax + P^T for one sub-pair; returns pkm2 tile."""
            P2 = p_bufs[pair_ctr[0] % NPBUF]
            pair_ctr[0] += 1
            pkm2 = pkpool.tile([128, 2, HEADS, 128], bf16, tag="pkm",
                               name="pkm")
            for j in range(2):
                s = sp * 2 + j
                cols = ts(s, 128)
                sc = ppsc.tile([128, 4, 512], f32, tag="sc", name="sc")
                for h in range(HEADS):
                    if h < 4:
                        qs = qkA[ts(h, 32), 0, cols]
                        ks = qkA[ts(h, 32), 1, cols]
                    else:
                        qs = qkB[ts(h - 4, 32), 0, cols]
                        ks = qkB[ts(h - 4, 32), 1, cols]
                    out = sc[:, h % 4, ds((h // 4) * 128, 128)]
                    nc.tensor.matmul(out, qs, ks,
                                     start=True, stop=True,
                                     tile_position=(32 * (h % 4), 0))
                E = apool.tile([128, HEADS, 128], bf16, tag="E", name="E")
                sums = stpool.tile([128, HEADS], f32, tag="sm", name="sums")
                rsum = stpool.tile([128, HEADS], f32, tag="rs", name="rsum")
                nc.scalar.activation(E[:, 0:4], sc[:, :, 0:128], AF.Exp)
                nc.scalar.activation(E[:, 4:6], sc[:, 0:2, 128:256], AF.Exp)
                nc.vector.reduce_sum(sums[0:64], E[0:64, :, 0:64],
                                     axis=mybir.AxisListType.X)
                nc.vector.reduce_sum(sums[64:128], E[64:128, :, 64:128],
                                     axis=mybir.AxisListType.X)
                nc.vector.reciprocal(rsum, sums)
                P = P2[:, j]
                for half in range(2):
                    hs = ds(half * 64, 64)
                    rs_half = rsum[ds(half * 64, 64)]
                    rsum_b = bass.AP(tensor=rs_half.tensor,
                                     offset=rs_half.offset,
                                     ap=[*rs_half.ap, [0, 64]])
                    eng = nc.vector if half == 0 else nc.gpsimd
                    eng.tensor_tensor(
                        out=P[hs, :, hs], in0=E[hs, :, hs],
                        in1=rsum_b, op=OP.mult)
            nc.sync.dma_start_transpose(out=pkm2, in_=P2)
            return pkm2

        def phase_b_out(sb, v_tm, pkm2, ofmA, ofmB, sp):
            """attention O matmuls + feature-major output copies."""
            oPp = ppm.tile([128, 2, 256], f32, tag="med", name="oPp")
            for j in range(2):
                s = sp * 2 + j
                for h in range(HEADS):
                    if h < 4:
                        out = oPp[ts(h, 32), j, 0:128]
                        colpos = h * 32
                    else:
                        out = oPp[ts(h - 4, 32), j, 128:256]
                        colpos = (h - 4) * 32
                    nc.tensor.matmul(out, v_tm[:, s, ts(h, 32)],
                                     pkm2[:, j, h], start=True, stop=True,
                                     tile_position=(0, colpos))
            if biases_zero:
                nc.vector.tensor_copy(
                    ofmA.rearrange("p (a b) -> p a b", a=NSUB)[:, ds(sp * 2, 2)],
                    oPp[:, :, 0:128])
                nc.vector.tensor_copy(
                    ofmB.rearrange("p (a b) -> p a b", a=NSUB)[:, ds(sp * 2, 2)],
                    oPp[0:64, :, 128:256])
            else:
                c0 = sp * 256
                for j in range(2):
                    nc.scalar.activation(
                        ofmA[:, ds(c0 + j * 128, 128)], oPp[:, j, 0:128],
                        AF.Identity, bias=sb["bv"][:, 0:1])
                    nc.scalar.activation(
                        ofmB[:, ds(c0 + j * 128, 128)],
                        oPp[0:64, j, 128:256],
                        AF.Identity, bias=sb["bv"][0:64, 1:2])

        def phase_c_proj(sb, x_t, ofmA, ofmB):
            """proj + residual."""
            for sp in range(2):
                psp = ppm.tile([128, 2, 256], f32, tag="med", name="psp")
                for j in range(2):
                    s = sp * 2 + j
                    nc.tensor.matmul(psp[:, j, 0:D], ofmA[:, ts(s, 128)],
                                     sb["wp0"], start=True, stop=False)
                    nc.tensor.matmul(psp[:, j, 0:D], ofmB[:, ts(s, 128)],
                                     sb["wp1"], start=False, stop=True)
                nc.vector.tensor_add(x_t[:, ds(sp * 2, 2), 0:D],
                                     x_t[:, ds(sp * 2, 2), 0:D],
                                     psp[:, :, 0:D])
                if not biases_zero:
                    for j in range(2):
                        nc.vector.tensor_add(x_t[:, sp * 2 + j, 0:D],
                                             x_t[:, sp * 2 + j, 0:D], sb["bp"])
        def phase_c_mlp(sb, x_t, ynA, ynA2):
            """MLP + residual."""
            ynAf = ynA.rearrange("p a b -> p (a b)")
            ynA2f = ynA2.rearrange("p a b -> p (a b)")
            hfm = hpool.tile([128, 6, TILE], bf16, tag="hfm", name="hfm")
            for m in range(6):
                psf = ppm.tile([128, TILE], f32, tag="med", name="psf1")
                nc.tensor.matmul(psf, sb["w10"][:, ts(m, 128)],
                                 ynAf, start=True, stop=False)
                nc.tensor.matmul(psf, sb["w11"][:, ts(m, 128)],
                                 ynA2f[0:64], start=False, stop=True)
                if biases_zero:
                    nc.scalar.activation(hfm[:, m], psf, GELU_FUNC)
                else:
                    nc.scalar.activation(hfm[:, m], psf, GELU_FUNC,
                                         bias=sb["b1"][:, m:m + 1])
            for sp in range(2):
                psf2 = ppm.tile([128, 2, 256], f32, tag="med", name="psf2")
                for j in range(2):
                    s = sp * 2 + j
                    for m in range(6):
                        nc.tensor.matmul(psf2[:, j, 0:D],
                                         hfm[:, m, ts(s, 128)],
                                         sb["w2m"][:, m],
                                         start=(m == 0), stop=(m == 5))
                nc.vector.tensor_add(x_t[:, ds(sp * 2, 2), 0:D],
                                     x_t[:, ds(sp * 2, 2), 0:D],
                                     psf2[:, :, 0:D])
                if not biases_zero:
                    for j in range(2):
                        nc.vector.tensor_add(x_t[:, sp * 2 + j, 0:D],
                                             x_t[:, sp * 2 + j, 0:D], sb["b2"])

        ngroups = (ntiles + GROUP - 1) // GROUP
        HOIST = 2

        def load_x(it):
            x_t = xpool.tile([128, NSUB, DP], f32, tag="x", name="x_t")
            nc.scalar.dma_start(
                out=x_t,
                in_=x_d[it].rearrange("p (s f) -> p s f", s=NSUB))
            return x_t

        carry_x = {}
        carry_fms = {}
        for g in range(ngroups):
            tiles = [g * GROUP + t for t in range(GROUP)
                     if g * GROUP + t < ntiles]
            xts = dict(carry_x)
            carry_x = {}
            for it in tiles:
                if it not in xts:
                    xts[it] = load_x(it)
            for li in range(DEPTH):
                sb = W[li]
                fms = {}
                for it in tiles:
                    if li == 0 and it in carry_fms:
                        fms[it] = carry_fms.pop(it)
                    else:
                        fms[it] = layernorm_fm(xts[it], "1")
                qk = {}
                for it in tiles:
                    qk[it] = phase_a(sb, *fms[it])
                of = {}
                for it in tiles:
                    of[it] = (ofpool.tile([128, TILE], bf16, tag="ofA",
                                          name="ofmA"),
                              ofpool.tile([64, TILE], bf16, tag="ofB",
                                          name="ofmB"))
                chains = [(it, sp) for it in tiles for sp in range(2)]
                SKEW = 5
                pk = {}
                for ci in range(len(chains) + SKEW):
                    if ci < len(chains):
                        it, sp = chains[ci]
                        qkA, qkB, _ = qk[it]
                        pk[ci] = phase_b_soft(sb, qkA, qkB, sp)
                    if ci >= SKEW:
                        it, sp = chains[ci - SKEW]
                        _, _, v_tm = qk[it]
                        phase_b_out(sb, v_tm, pk.pop(ci - SKEW),
                                    of[it][0], of[it][1], sp)
                for it in tiles:
                    ofmA, ofmB = of[it]
                    phase_c_proj(sb, xts[it], ofmA, ofmB)
                yns = {}
                for it in tiles:
                    yns[it] = layernorm_fm(xts[it], "2")
                if li == DEPTH - 1 and g + 1 < ngroups:
                    for nt in range(HOIST):
                        it2 = (g + 1) * GROUP + nt
                        if it2 < ntiles:
                            carry_x[it2] = load_x(it2)
                    for it2 in list(carry_x):
                        carry_fms[it2] = layernorm_fm(carry_x[it2], "1")
                for it in tiles:
                    phase_c_mlp(sb, xts[it], *yns[it])
            for it in tiles:
                nc.scalar.dma_start(
                    out=y_d[it].rearrange("p (s f) -> p s f", s=NSUB),
                    in_=xts[it][:, :, 0:D])

    nc.compile()
    _COMPILED[key] = nc
    return nc


def _ensure_ntff_hook():
    import sys, types
    if "antenv.axon_hooks" in sys.modules:
        return True
    try:
        mod = types.ModuleType("antenv.axon_hooks")
        state = {}
        mod.set_axon_ntff_profile_hook = lambda h: state.__setitem__("h", h)
        mod.get_axon_ntff_profile_hook = lambda: state.get("h")
        sys.modules["antenv.axon_hooks"] = mod
        import antenv
        antenv.axon_hooks = mod
        from trn_agent_boot.trn_boot import _ntff_profile_via_ctypes
        mod.set_axon_ntff_profile_hook(
            _ntff_profile_via_ctypes("/opt/axon/libaxon_pjrt.so"))
        return True
    except Exception as e:  # pragma: no cover
        print(f"NTFF hook shim failed: {e}")
        return False


def _run(inputs, trace=False):
    """Shard, execute on 8 cores, gather. Returns (y_full, exec_time_ns)."""
    from concourse.bass_utils import run_bass_kernel_spmd

    if trace:
        trace = _ensure_ntff_hook()

    layers = _fold_weights(inputs)
    bz = all(
        not np.any(np.asarray(d[k], np.float32))
        for d in layers for k in d if k.startswith(("bqk", "bv", "bp", "b1", "b2")))
    nc = _build_nc(biases_zero=bz)

    x = np.asarray(inputs["x"], np.float32)
    pos = np.asarray(inputs["pos"], np.float32)
    w = int(np.asarray(inputs["w"]))
    order = _scanline_order(pos, w)
    x_ord = np.take_along_axis(x, order[..., None], axis=1)
    # device layout: [NTILES, 128 (token-in-sub), NSUB, DP]
    sw = np.zeros((NCORES, NTILES, 128, NSUB, DP), np.float32)
    sw[..., 0:D] = x_ord.reshape(NCORES, NTILES, NSUB, 128, D).transpose(
        0, 1, 3, 2, 4)
    shards = sw.reshape(NCORES, NTILES, 128, NSUB * DP)

    wmap = {}
    for d in layers:
        wmap.update({k: np.ascontiguousarray(v) for k, v in d.items()})

    in_maps = [{"x": shards[c], **wmap} for c in range(NCORES)]
    res = run_bass_kernel_spmd(nc, in_maps, core_ids=list(range(NCORES)),
                               trace=trace)
    y_ord = np.stack([res.results[c]["y"] for c in range(NCORES)])
    y_ord = y_ord.reshape(NCORES, NTILES, 128, NSUB, D).transpose(
        0, 1, 3, 2, 4)
    y_ord = y_ord.reshape(B, N, D)
    y = np.empty_like(y_ord)
    np.put_along_axis(y, order[..., None], y_ord, axis=1)
    return y.astype(np.float32), res.exec_time_ns


def kernel(**inputs):
    y, _ = _run(inputs, trace=False)
    return y


# revision 20
# speedup vs baseline: 1.0099x; 1.0099x over previous
"""Trainium2 Bass kernel for nn_BasicLayer (sparse cluster attention, 2 layers).

v2 rewrite of the staged baseline. Same host-side strategy (scanline gather,
8 cores x 8192 tokens, folded weights, token-major fp32 residual, bf16 matmul
operands) with an on-device restructure aimed at engine balance and PE
density:

- All layout flips (LN token-major -> feature-major, P -> P^T) go through the
  DMA xbar transpose engine instead of PE transpose + PSUM copy.
- LN: one batched bn_stats pair, rsqrt via fast-inverse-sqrt bit trick +
  1 Newton step (DVE only, no sqrt table), normalize via dual-scalar
  tensor_scalar producing bf16 directly.
- Scores matmuls read per-head q/k slices in place via tile_position row
  packing (no per-head copies).
- Softmax: exp -> scratch E; P = E * (1/rowsum) only on the diagonal
  64x64 blocks into persistent zeroed P buffers (gpsimd); P^T via DMA
  transpose feeds the O matmuls.
- Supertiles processed in groups of 4 with phase-major ordering per layer so
  the scalar engine's activation-table switches (Exp <-> Gelu) amortize
  across the group.
"""

import os
import numpy as np
import ml_dtypes

# ---- problem constants (hardcoded per contract) ----
B, N, D = 4, 16384, 192
DP = 256
HEADS, DH, CLM = 6, 32, 64
GRID_W = 128
DEPTH = 2
NCORES = 8
T = (B * N) // NCORES                # 8192 tokens per core
SUB = 128
NSUB = 4
TILE = SUB * NSUB                    # 512-token supertile
NTILES = T // TILE                   # 16
GROUP = 8                            # supertiles per phase group
DFF = 768

_COMPILED = {}


def _scanline_order(pos, w):
    ix = np.floor(pos[..., 0]).astype(np.int64)
    iy = np.floor(pos[..., 1]).astype(np.int64)
    key = iy * w + np.where(iy % 2 == 1, w - 1 - ix, ix)
    return np.argsort(key, axis=1, kind="stable")


def _fold_weights(inputs):
    """Fold LN affine + biases into matmul weights (same layout as v1)."""
    bf16 = ml_dtypes.bfloat16
    scale = DH ** -0.5
    layers = []
    for i in range(DEPTH):
        g1 = np.asarray(inputs["ln1_g"][i], np.float64)
        b1 = np.asarray(inputs["ln1_b"][i], np.float64)
        Wqkv = np.asarray(inputs["w_qkv"][i], np.float64)
        bqkv = np.asarray(inputs["b_qkv"][i], np.float64)
        w_eff = g1[:, None] * Wqkv
        b_eff = b1 @ Wqkv + bqkv
        wq = w_eff[:, 0:D] * scale
        bq = b_eff[0:D] * scale
        wk = w_eff[:, D:2 * D]
        bk = b_eff[D:2 * D]
        wv = w_eff[:, 2 * D:3 * D]
        bv = b_eff[2 * D:3 * D]
        wqk = np.concatenate(
            [wq[:, :128], wk[:, :128], wq[:, 128:], wk[:, 128:]], axis=1)
        pad64 = np.zeros(64)
        bqk = np.stack(
            [bq[:128], bk[:128],
             np.concatenate([bq[128:], pad64]),
             np.concatenate([bk[128:], pad64])], axis=1)
        wp = np.asarray(inputs["w_proj"][i], np.float64)
        bp = np.asarray(inputs["b_proj"][i], np.float64)
        g2 = np.asarray(inputs["ln2_g"][i], np.float64)
        b2 = np.asarray(inputs["ln2_b"][i], np.float64)
        W1 = np.asarray(inputs["w_fc1"][i], np.float64)
        w1_eff = g2[:, None] * W1
        b1_eff = b2 @ W1 + np.asarray(inputs["b_fc1"][i], np.float64)
        W2 = np.asarray(inputs["w_fc2"][i], np.float64)
        bfc2 = np.asarray(inputs["b_fc2"][i], np.float64)
        bv_t = np.stack(
            [bv[:128], np.concatenate([bv[128:], np.zeros(64)])], axis=1)
        layers.append({
            f"wqk{i}": wqk.astype(bf16),
            f"bqk{i}": bqk.astype(np.float32),
            f"wv{i}": wv.astype(bf16),
            f"bv{i}": bv_t.astype(np.float32),
            f"wp{i}": wp.astype(bf16),
            f"bp{i}": np.tile(bp.astype(np.float32), (128, 1)),
            f"w1{i}": w1_eff.astype(bf16),
            f"b1{i}": b1_eff.reshape(6, 128).T.copy().astype(np.float32),
            f"w2{i}": W2.astype(bf16),
            f"b2{i}": np.tile(bfc2.astype(np.float32), (128, 1)),
        })
    return layers


def _build_nc(biases_zero=True, ntiles=NTILES):
    key = ("nc", biases_zero, ntiles)
    if key in _COMPILED:
        return _COMPILED[key]

    from contextlib import ExitStack
    import concourse.bass as bass
    import concourse.tile as tile
    from concourse import bacc, mybir
    from concourse.bass import ts, ds

    f32 = mybir.dt.float32
    bf16 = mybir.dt.bfloat16
    i32 = mybir.dt.int32
    AF = mybir.ActivationFunctionType
    OP = mybir.AluOpType

    tok_total = ntiles * TILE

    nc = bacc.Bacc("TRN2", target_bir_lowering=False, debug=False,
                   enable_asserts=False, num_devices=NCORES)

    x_d = nc.dram_tensor("x", [ntiles, 128, NSUB * DP], f32,
                         kind="ExternalInput").ap()
    y_d = nc.dram_tensor("y", [ntiles, 128, NSUB * D], f32,
                         kind="ExternalOutput").ap()
    wd = []
    for i in range(DEPTH):
        wd.append({
            "wqk": nc.dram_tensor(f"wqk{i}", [D, 384], bf16, kind="ExternalInput").ap(),
            "bqk": nc.dram_tensor(f"bqk{i}", [128, 4], f32, kind="ExternalInput").ap(),
            "wv": nc.dram_tensor(f"wv{i}", [D, D], bf16, kind="ExternalInput").ap(),
            "bv": nc.dram_tensor(f"bv{i}", [128, 2], f32, kind="ExternalInput").ap(),
            "wp": nc.dram_tensor(f"wp{i}", [D, D], bf16, kind="ExternalInput").ap(),
            "bp": nc.dram_tensor(f"bp{i}", [128, D], f32, kind="ExternalInput").ap(),
            "w1": nc.dram_tensor(f"w1{i}", [D, DFF], bf16, kind="ExternalInput").ap(),
            "b1": nc.dram_tensor(f"b1{i}", [128, 6], f32, kind="ExternalInput").ap(),
            "w2": nc.dram_tensor(f"w2{i}", [DFF, D], bf16, kind="ExternalInput").ap(),
            "b2": nc.dram_tensor(f"b2{i}", [128, D], f32, kind="ExternalInput").ap(),
        })

    with tile.TileContext(nc) as tc, ExitStack() as ctx:
        consts = ctx.enter_context(tc.tile_pool(name="consts", bufs=1))
        xpool = ctx.enter_context(tc.tile_pool(name="xpool", bufs=10))
        lnpool = ctx.enter_context(tc.tile_pool(name="lnpool", bufs=6))
        fmpool = ctx.enter_context(tc.tile_pool(name="fmpool", bufs=10))
        qkpool = ctx.enter_context(tc.tile_pool(name="qkpool", bufs=8))
        apool = ctx.enter_context(tc.tile_pool(name="apool", bufs=3))
        pkpool = ctx.enter_context(tc.tile_pool(name="pkpool", bufs=7))
        ofpool = ctx.enter_context(tc.tile_pool(name="ofpool", bufs=8))
        hpool = ctx.enter_context(tc.tile_pool(name="hpool", bufs=2))
        stpool = ctx.enter_context(tc.tile_pool(name="stpool", bufs=9))
        ppsc = ctx.enter_context(tc.tile_pool(name="ppsc", bufs=1, space="PSUM"))
        ppm = ctx.enter_context(tc.tile_pool(name="ppm", bufs=4, space="PSUM"))

        # persistent softmax buffers: off-diagonal blocks stay 0 forever
        NPBUF = 4
        p_bufs = []
        for pb_i in range(NPBUF):
            pb = consts.tile([128, 2, HEADS, 128], bf16, name=f"pbuf{pb_i}")
            nc.vector.memset(pb, 0.0)
            p_bufs.append(pb)

        # --- load weights into SBUF once ---
        W = []
        for i in range(DEPTH):
            d = wd[i]
            sb = {}
            sb["wqk0"] = consts.tile([128, 384], bf16, name=f"wqk0{i}")
            sb["wqk1"] = consts.tile([64, 384], bf16, name=f"wqk1{i}")
            nc.scalar.dma_start(out=sb["wqk0"], in_=d["wqk"][0:128])
            nc.scalar.dma_start(out=sb["wqk1"], in_=d["wqk"][128:192])
            sb["wv0"] = consts.tile([128, D], bf16, name=f"wv0{i}")
            sb["wv1"] = consts.tile([64, D], bf16, name=f"wv1{i}")
            nc.scalar.dma_start(out=sb["wv0"], in_=d["wv"][0:128])
            nc.scalar.dma_start(out=sb["wv1"], in_=d["wv"][128:192])
            sb["wp0"] = consts.tile([128, D], bf16, name=f"wp0{i}")
            sb["wp1"] = consts.tile([64, D], bf16, name=f"wp1{i}")
            nc.scalar.dma_start(out=sb["wp0"], in_=d["wp"][0:128])
            nc.scalar.dma_start(out=sb["wp1"], in_=d["wp"][128:192])
            sb["w10"] = consts.tile([128, DFF], bf16, name=f"w10{i}")
            sb["w11"] = consts.tile([64, DFF], bf16, name=f"w11{i}")
            nc.scalar.dma_start(out=sb["w10"], in_=d["w1"][0:128])
            nc.scalar.dma_start(out=sb["w11"], in_=d["w1"][128:192])
            sb["w2m"] = consts.tile([128, 6, D], bf16, name=f"w2m{i}")
            nc.scalar.dma_start(
                out=sb["w2m"],
                in_=d["w2"].rearrange("(m p) n -> p m n", p=128))
            for nm in ("bqk", "bv", "b1", "bp", "b2"):
                shp = {"bqk": [128, 4], "bv": [128, 2], "b1": [128, 6],
                       "bp": [128, D], "b2": [128, D]}[nm]
                sb[nm] = consts.tile(shp, f32, name=f"{nm}{i}")
                nc.scalar.dma_start(out=sb[nm], in_=d[nm])
            W.append(sb)

        pair_ctr = [0]
        MAGIC = 0x5F3759DF
        # CoreSim lacks Gelu_apprx_tanh; substitute Tanh for sim-only runs.
        GELU_FUNC = (AF.Tanh if os.environ.get("K_SIM_GELU_TANH") == "1"
                     else AF.Gelu_apprx_tanh)

        def layernorm_fm(x_t, tag):
            """LN on token-major x_t -> feature-major bf16 via DMA transpose.
            Returns fmA [128,4,128] (feats 0:128, cols=tokens) and fmA2
            (feats 128:256; partitions 64:128 are pad)."""
            mv = stpool.tile([128, 4, 6], f32, tag="mv", name="mv")
            mv2 = stpool.tile([128, 4, 2], f32, tag="mv2", name="mv2")
            for s in range(NSUB):
                nc.vector.bn_stats(mv[:, s], x_t[:, s, 0:D])
                nc.vector.bn_aggr(mv2[:, s], mv[:, s])
            var = mv2[:, :, 1]                       # [128, 4] stride 2
            t_i = stpool.tile([128, 4], i32, tag="ti", name="t_i")
            y0 = stpool.tile([128, 4], f32, tag="y0", name="y0")
            zz = stpool.tile([128, 4], f32, tag="zz", name="zz")
            r4 = stpool.tile([128, 4], f32, tag="r4", name="r4")
            nc.vector.tensor_scalar(
                out=t_i, in0=var.bitcast(i32), scalar1=1, scalar2=None,
                op0=OP.logical_shift_right)
            nc.vector.tensor_scalar(
                out=y0.bitcast(i32), in0=t_i, scalar1=MAGIC, scalar2=-1,
                op0=OP.subtract, op1=OP.mult)
            nc.vector.scalar_tensor_tensor(
                out=zz, in0=var, scalar=1e-5, in1=y0,
                op0=OP.add, op1=OP.mult)              # (var+eps)*y0
            nc.vector.tensor_tensor(out=zz, in0=zz, in1=y0, op=OP.mult)
            nc.vector.tensor_scalar(
                out=zz, in0=zz, scalar1=-0.5, scalar2=1.5,
                op0=OP.mult, op1=OP.add)              # 1.5 - 0.5 v y0^2
            nc.vector.tensor_tensor(out=r4, in0=zz, in1=y0, op=OP.mult)

            xn = lnpool.tile([128, 2, NSUB, 128], bf16, tag="xn",
                             name=f"xn{tag}")
            for s in range(NSUB):
                nc.vector.tensor_scalar(
                    out=xn[:, :, s], in0=x_t[:, s].rearrange("p (c f) -> p c f", c=2),
                    scalar1=mv2[:, s, 0:1], scalar2=r4[:, s:s + 1],
                    op0=OP.subtract, op1=OP.mult)
            fm2 = fmpool.tile([128, 2, NSUB, 128], bf16, tag="fm",
                              name=f"fm{tag}")
            nc.sync.dma_start_transpose(out=fm2, in_=xn)
            return fm2[:, 0], fm2[:, 1]

        def phase_a(sb, fmA, fmA2):
            """qkv + v from feature-major LN output. Returns (qkA, qkB, v_tm)."""
            fmAf = fmA.rearrange("p a b -> p (a b)")
            fmA2f = fmA2.rearrange("p a b -> p (a b)")
            # qkv: m-chunks 0,1 are 128-wide (heads 0-3 q|k), 2,3 are 64-wide
            qkA = qkpool.tile([128, 2, TILE], bf16, tag="qkA", name="qkA")
            qkB = qkpool.tile([64, 2, TILE], bf16, tag="qkB", name="qkB")
            psq = []
            for m in range(2):
                ps = ppm.tile([128, TILE], f32, tag="med", name=f"psqA{m}")
                nc.tensor.matmul(ps, sb["wqk0"][:, ts(m, 128)], fmAf,
                                 start=True, stop=False)
                nc.tensor.matmul(ps, sb["wqk1"][:, ts(m, 128)],
                                 fmA2f[0:64], start=False, stop=True)
                psq.append(ps)
            for m in range(2):
                ps = ppm.tile([64, TILE], f32, tag="med", name=f"psqB{m}")
                nc.tensor.matmul(ps, sb["wqk0"][:, ds(256 + m * 64, 64)],
                                 fmAf, start=True, stop=False)
                nc.tensor.matmul(ps, sb["wqk1"][:, ds(256 + m * 64, 64)],
                                 fmA2f[0:64], start=False, stop=True)
                psq.append(ps)
            for m in range(2):
                if biases_zero:
                    nc.scalar.activation(qkA[:, m], psq[m], AF.Copy)
                    nc.vector.tensor_copy(qkB[:, m], psq[2 + m])
                else:
                    nc.scalar.activation(qkA[:, m], psq[m], AF.Identity,
                                         bias=sb["bqk"][:, m:m + 1])
                    nc.scalar.activation(qkB[:, m], psq[2 + m], AF.Identity,
                                         bias=sb["bqk"][0:64, 2 + m:3 + m])
            # v (token-major out), pairs of subs per PSUM tile
            v_tm = qkpool.tile([128, NSUB, D], bf16, tag="vtm", name="v_tm")
            for sp in range(2):
                psv = ppm.tile([128, 2, 256], f32, tag="med", name="psv")
                for j in range(2):
                    s = sp * 2 + j
                    nc.tensor.matmul(psv[:, j, 0:D], fmA[:, s], sb["wv0"],
                                     start=True, stop=False)
                    nc.tensor.matmul(psv[:, j, 0:D], fmA2[0:64, s], sb["wv1"],
                                     start=False, stop=True)
                if biases_zero:
                    nc.scalar.activation(v_tm[:, ds(sp * 2, 2)],
                                         psv[:, :, 0:D], AF.Copy)
                else:
                    for j in range(2):
                        nc.scalar.activation(
                            v_tm[:, sp * 2 + j], psv[:, j, 0:D], AF.Identity,
                            bias=sb["bv"][:, 0:1])
            return qkA, qkB, v_tm

        def phase_b_soft(sb, qkA, qkB, sp):
            """scores + softmax + P^T for one sub-pair; returns pkm2 tile."""
            P2 = p_bufs[pair_ctr[0] % NPBUF]
            pair_ctr[0] += 1
            pkm2 = pkpool.tile([128, 2, HEADS, 128], bf16, tag="pkm",
                               name="pkm")
            for j in range(2):
                s = sp * 2 + j
                cols = ts(s, 128)
                sc = ppsc.tile([128, 4, 512], f32, tag="sc", name="sc")
                for h in range(HEADS):
                    if h < 4:
                        qs = qkA[ts(h, 32), 0, cols]
                        ks = qkA[ts(h, 32), 1, cols]
                    else:
                        qs = qkB[ts(h - 4, 32), 0, cols]
                        ks = qkB[ts(h - 4, 32), 1, cols]
                    out = sc[:, h % 4, ds((h // 4) * 128, 128)]
                    nc.tensor.matmul(out, qs, ks,
                                     start=True, stop=True,
                                     tile_position=(32 * (h % 4), 0))
                E = apool.tile([128, HEADS, 128], bf16, tag="E", name="E")
                sums = stpool.tile([128, HEADS], f32, tag="sm", name="sums")
                rsum = stpool.tile([128, HEADS], f32, tag="rs", name="rsum")
                nc.scalar.activation(E[:, 0:4], sc[:, :, 0:128], AF.Exp)
                nc.scalar.activation(E[:, 4:6], sc[:, 0:2, 128:256], AF.Exp)
                nc.vector.reduce_sum(sums[0:64], E[0:64, :, 0:64],
                                     axis=mybir.AxisListType.X)
                nc.vector.reduce_sum(sums[64:128], E[64:128, :, 64:128],
                                     axis=mybir.AxisListType.X)
                nc.vector.reciprocal(rsum, sums)
                P = P2[:, j]
                for half in range(2):
                    hs = ds(half * 64, 64)
                    rs_half = rsum[ds(half * 64, 64)]
                    rsum_b = bass.AP(tensor=rs_half.tensor,
                                     offset=rs_half.offset,
                                     ap=[*rs_half.ap, [0, 64]])
                    eng = nc.vector if half == 0 else nc.gpsimd
                    eng.tensor_tensor(
                        out=P[hs, :, hs], in0=E[hs, :, hs],
                        in1=rsum_b, op=OP.mult)
            nc.sync.dma_start_transpose(out=pkm2, in_=P2)
            return pkm2

        def phase_b_out(sb, v_tm, pkm2, ofmA, ofmB, sp):
            """attention O matmuls + feature-major output copies."""
            oPp = ppm.tile([128, 2, 256], f32, tag="med", name="oPp")
            for j in range(2):
                s = sp * 2 + j
                for h in range(HEADS):
                    if h < 4:
                        out = oPp[ts(h, 32), j, 0:128]
                        colpos = h * 32
                    else:
                        out = oPp[ts(h - 4, 32), j, 128:256]
                        colpos = (h - 4) * 32
                    nc.tensor.matmul(out, v_tm[:, s, ts(h, 32)],
                                     pkm2[:, j, h], start=True, stop=True,
                                     tile_position=(0, colpos))
            if biases_zero:
                nc.vector.tensor_copy(
                    ofmA.rearrange("p (a b) -> p a b", a=NSUB)[:, ds(sp * 2, 2)],
                    oPp[:, :, 0:128])
                nc.vector.tensor_copy(
                    ofmB.rearrange("p (a b) -> p a b", a=NSUB)[:, ds(sp * 2, 2)],
                    oPp[0:64, :, 128:256])
            else:
                c0 = sp * 256
                for j in range(2):
                    nc.scalar.activation(
                        ofmA[:, ds(c0 + j * 128, 128)], oPp[:, j, 0:128],
                        AF.Identity, bias=sb["bv"][:, 0:1])
                    nc.scalar.activation(
                        ofmB[:, ds(c0 + j * 128, 128)],
                        oPp[0:64, j, 128:256],
                        AF.Identity, bias=sb["bv"][0:64, 1:2])

        def phase_c_proj(sb, x_t, ofmA, ofmB):
            """proj + residual."""
            for sp in range(2):
                psp = ppm.tile([128, 2, 256], f32, tag="med", name="psp")
                for j in range(2):
                    s = sp * 2 + j
                    nc.tensor.matmul(psp[:, j, 0:D], ofmA[:, ts(s, 128)],
                                     sb["wp0"], start=True, stop=False)
                    nc.tensor.matmul(psp[:, j, 0:D], ofmB[:, ts(s, 128)],
                                     sb["wp1"], start=False, stop=True)
                nc.vector.tensor_add(x_t[:, ds(sp * 2, 2), 0:D],
                                     x_t[:, ds(sp * 2, 2), 0:D],
                                     psp[:, :, 0:D])
                if not biases_zero:
                    for j in range(2):
                        nc.vector.tensor_add(x_t[:, sp * 2 + j, 0:D],
                                             x_t[:, sp * 2 + j, 0:D], sb["bp"])
        def phase_c_mlp(sb, x_t, ynA, ynA2):
            """MLP + residual."""
            ynAf = ynA.rearrange("p a b -> p (a b)")
            ynA2f = ynA2.rearrange("p a b -> p (a b)")
            hfm = hpool.tile([128, 6, TILE], bf16, tag="hfm", name="hfm")
            for m in range(6):
                psf = ppm.tile([128, TILE], f32, tag="med", name="psf1")
                nc.tensor.matmul(psf, sb["w10"][:, ts(m, 128)],
                                 ynAf, start=True, stop=False)
                nc.tensor.matmul(psf, sb["w11"][:, ts(m, 128)],
                                 ynA2f[0:64], start=False, stop=True)
                if biases_zero:
                    nc.scalar.activation(hfm[:, m], psf, GELU_FUNC)
                else:
                    nc.scalar.activation(hfm[:, m], psf, GELU_FUNC,
                                         bias=sb["b1"][:, m:m + 1])
            for sp in range(2):
                psf2 = ppm.tile([128, 2, 256], f32, tag="med", name="psf2")
                for j in range(2):
                    s = sp * 2 + j
                    for m in range(6):
                        nc.tensor.matmul(psf2[:, j, 0:D],
                                         hfm[:, m, ts(s, 128)],
                                         sb["w2m"][:, m],
                                         start=(m == 0), stop=(m == 5))
                nc.vector.tensor_add(x_t[:, ds(sp * 2, 2), 0:D],
                                     x_t[:, ds(sp * 2, 2), 0:D],
                                     psf2[:, :, 0:D])
                if not biases_zero:
                    for j in range(2):
                        nc.vector.tensor_add(x_t[:, sp * 2 + j, 0:D],
                                             x_t[:, sp * 2 + j, 0:D], sb["b2"])

        ngroups = (ntiles + GROUP - 1) // GROUP
        HOIST = 2

        def load_x(it):
            x_t = xpool.tile([128, NSUB, DP], f32, tag="x", name="x_t")
            nc.sync.dma_start(
                out=x_t,
                in_=x_d[it].rearrange("p (s f) -> p s f", s=NSUB))
            return x_t

        carry_x = {}
        carry_fms = {}
        for g in range(ngroups):
            tiles = [g * GROUP + t for t in range(GROUP)
                     if g * GROUP + t < ntiles]
            xts = dict(carry_x)
            carry_x = {}
            for it in tiles:
                if it not in xts:
                    xts[it] = load_x(it)
            for li in range(DEPTH):
                sb = W[li]
                fms = {}
                for it in tiles:
                    if li == 0 and it in carry_fms:
                        fms[it] = carry_fms.pop(it)
                    else:
                        fms[it] = layernorm_fm(xts[it], "1")
                qk = {}
                for it in tiles:
                    qk[it] = phase_a(sb, *fms[it])
                of = {}
                for it in tiles:
                    of[it] = (ofpool.tile([128, TILE], bf16, tag="ofA",
                                          name="ofmA"),
                              ofpool.tile([64, TILE], bf16, tag="ofB",
                                          name="ofmB"))
                chains = [(it, sp) for it in tiles for sp in range(2)]
                SKEW = 5
                pk = {}
                for ci in range(len(chains) + SKEW):
                    if ci < len(chains):
                        it, sp = chains[ci]
                        qkA, qkB, _ = qk[it]
                        pk[ci] = phase_b_soft(sb, qkA, qkB, sp)
                    if ci >= SKEW:
                        it, sp = chains[ci - SKEW]
                        _, _, v_tm = qk[it]
                        phase_b_out(sb, v_tm, pk.pop(ci - SKEW),
                                    of[it][0], of[it][1], sp)
                for it in tiles:
                    ofmA, ofmB = of[it]
                    phase_c_proj(sb, xts[it], ofmA, ofmB)
                yns = {}
                for it in tiles:
                    yns[it] = layernorm_fm(xts[it], "2")
                if li == DEPTH - 1 and g + 1 < ngroups:
                    for nt in range(HOIST):
                        it2 = (g + 1) * GROUP + nt
                        if it2 < ntiles:
                            carry_x[it2] = load_x(it2)
                    for it2 in list(carry_x):
                        carry_fms[it2] = layernorm_fm(carry_x[it2], "1")
                for it in tiles:
                    phase_c_mlp(sb, xts[it], *yns[it])
            for it in tiles:
                nc.sync.dma_start(
                    out=y_d[it].rearrange("p (s f) -> p s f", s=NSUB),
                    in_=xts[it][:, :, 0:D])

    nc.compile()
    _COMPILED[key] = nc
    return nc


def _ensure_ntff_hook():
    import sys, types
    if "antenv.axon_hooks" in sys.modules:
        return True
    try:
        mod = types.ModuleType("antenv.axon_hooks")
        state = {}
        mod.set_axon_ntff_profile_hook = lambda h: state.__setitem__("h", h)
        mod.get_axon_ntff_profile_hook = lambda: state.get("h")
        sys.modules["antenv.axon_hooks"] = mod
        import antenv
        antenv.axon_hooks = mod
        from trn_agent_boot.trn_boot import _ntff_profile_via_ctypes
        mod.set_axon_ntff_profile_hook(
            _ntff_profile_via_ctypes("/opt/axon/libaxon_pjrt.so"))
        return True
    except Exception as e:  # pragma: no cover
        print(f"NTFF hook shim failed: {e}")
        return False


def _run(inputs, trace=False):
    """Shard, execute on 8 cores, gather. Returns (y_full, exec_time_ns)."""
    from concourse.bass_utils import run_bass_kernel_spmd

    if trace:
        trace = _ensure_ntff_hook()

    layers = _fold_weights(inputs)
    bz = all(
        not np.any(np.asarray(d[k], np.float32))
        for d in layers for k in d if k.startswith(("bqk", "bv", "bp", "b1", "b2")))
    nc = _build_nc(biases_zero=bz)

    x = np.asarray(inputs["x"], np.float32)
    pos = np.asarray(inputs["pos"], np.float32)
    w = int(np.asarray(inputs["w"]))
    order = _scanline_order(pos, w)
    x_ord = np.take_along_axis(x, order[..., None], axis=1)
    # device layout: [NTILES, 128 (token-in-sub), NSUB, DP]
    sw = np.zeros((NCORES, NTILES, 128, NSUB, DP), np.float32)
    sw[..., 0:D] = x_ord.reshape(NCORES, NTILES, NSUB, 128, D).transpose(
        0, 1, 3, 2, 4)
    shards = sw.reshape(NCORES, NTILES, 128, NSUB * DP)

    wmap = {}
    for d in layers:
        wmap.update({k: np.ascontiguousarray(v) for k, v in d.items()})

    in_maps = [{"x": shards[c], **wmap} for c in range(NCORES)]
    res = run_bass_kernel_spmd(nc, in_maps, core_ids=list(range(NCORES)),
                               trace=trace)
    y_ord = np.stack([res.results[c]["y"] for c in range(NCORES)])
    y_ord = y_ord.reshape(NCORES, NTILES, 128, NSUB, D).transpose(
        0, 1, 3, 2, 4)
    y_ord = y_ord.reshape(B, N, D)
    y = np.empty_like(y_ord)
    np.put_along_axis(y, order[..., None], y_ord, axis=1)
    return y.astype(np.float32), res.exec_time_ns


def kernel(**inputs):
    y, _ = _run(inputs, trace=False)
    return y


# revision 21
# speedup vs baseline: 1.0147x; 1.0048x over previous
"""Trainium2 Bass kernel for nn_BasicLayer (sparse cluster attention, 2 layers).

v2 rewrite of the staged baseline. Same host-side strategy (scanline gather,
8 cores x 8192 tokens, folded weights, token-major fp32 residual, bf16 matmul
operands) with an on-device restructure aimed at engine balance and PE
density:

- All layout flips (LN token-major -> feature-major, P -> P^T) go through the
  DMA xbar transpose engine instead of PE transpose + PSUM copy.
- LN: one batched bn_stats pair, rsqrt via fast-inverse-sqrt bit trick +
  1 Newton step (DVE only, no sqrt table), normalize via dual-scalar
  tensor_scalar producing bf16 directly.
- Scores matmuls read per-head q/k slices in place via tile_position row
  packing (no per-head copies).
- Softmax: exp -> scratch E; P = E * (1/rowsum) only on the diagonal
  64x64 blocks into persistent zeroed P buffers (gpsimd); P^T via DMA
  transpose feeds the O matmuls.
- Supertiles processed in groups of 4 with phase-major ordering per layer so
  the scalar engine's activation-table switches (Exp <-> Gelu) amortize
  across the group.
"""

import os
import numpy as np
import ml_dtypes

# ---- problem constants (hardcoded per contract) ----
B, N, D = 4, 16384, 192
DP = 256
HEADS, DH, CLM = 6, 32, 64
GRID_W = 128
DEPTH = 2
NCORES = 8
T = (B * N) // NCORES                # 8192 tokens per core
SUB = 128
NSUB = 4
TILE = SUB * NSUB                    # 512-token supertile
NTILES = T // TILE                   # 16
GROUP = 8                            # supertiles per phase group
DFF = 768

_COMPILED = {}


def _scanline_order(pos, w):
    ix = np.floor(pos[..., 0]).astype(np.int64)
    iy = np.floor(pos[..., 1]).astype(np.int64)
    key = iy * w + np.where(iy % 2 == 1, w - 1 - ix, ix)
    return np.argsort(key, axis=1, kind="stable")


def _fold_weights(inputs):
    """Fold LN affine + biases into matmul weights (same layout as v1)."""
    bf16 = ml_dtypes.bfloat16
    scale = DH ** -0.5
    layers = []
    for i in range(DEPTH):
        g1 = np.asarray(inputs["ln1_g"][i], np.float64)
        b1 = np.asarray(inputs["ln1_b"][i], np.float64)
        Wqkv = np.asarray(inputs["w_qkv"][i], np.float64)
        bqkv = np.asarray(inputs["b_qkv"][i], np.float64)
        w_eff = g1[:, None] * Wqkv
        b_eff = b1 @ Wqkv + bqkv
        wq = w_eff[:, 0:D] * scale
        bq = b_eff[0:D] * scale
        wk = w_eff[:, D:2 * D]
        bk = b_eff[D:2 * D]
        wv = w_eff[:, 2 * D:3 * D]
        bv = b_eff[2 * D:3 * D]
        wqk = np.concatenate(
            [wq[:, :128], wk[:, :128], wq[:, 128:], wk[:, 128:]], axis=1)
        pad64 = np.zeros(64)
        bqk = np.stack(
            [bq[:128], bk[:128],
             np.concatenate([bq[128:], pad64]),
             np.concatenate([bk[128:], pad64])], axis=1)
        wp = np.asarray(inputs["w_proj"][i], np.float64)
        bp = np.asarray(inputs["b_proj"][i], np.float64)
        g2 = np.asarray(inputs["ln2_g"][i], np.float64)
        b2 = np.asarray(inputs["ln2_b"][i], np.float64)
        W1 = np.asarray(inputs["w_fc1"][i], np.float64)
        w1_eff = g2[:, None] * W1
        b1_eff = b2 @ W1 + np.asarray(inputs["b_fc1"][i], np.float64)
        W2 = np.asarray(inputs["w_fc2"][i], np.float64)
        bfc2 = np.asarray(inputs["b_fc2"][i], np.float64)
        bv_t = np.stack(
            [bv[:128], np.concatenate([bv[128:], np.zeros(64)])], axis=1)
        layers.append({
            f"wqk{i}": wqk.astype(bf16),
            f"bqk{i}": bqk.astype(np.float32),
            f"wv{i}": wv.astype(bf16),
            f"bv{i}": bv_t.astype(np.float32),
            f"wp{i}": wp.astype(bf16),
            f"bp{i}": np.tile(bp.astype(np.float32), (128, 1)),
            f"w1{i}": w1_eff.astype(bf16),
            f"b1{i}": b1_eff.reshape(6, 128).T.copy().astype(np.float32),
            f"w2{i}": W2.astype(bf16),
            f"b2{i}": np.tile(bfc2.astype(np.float32), (128, 1)),
        })
    return layers


def _build_nc(biases_zero=True, ntiles=NTILES):
    key = ("nc", biases_zero, ntiles)
    if key in _COMPILED:
        return _COMPILED[key]

    from contextlib import ExitStack
    import concourse.bass as bass
    import concourse.tile as tile
    from concourse import bacc, mybir
    from concourse.bass import ts, ds

    f32 = mybir.dt.float32
    bf16 = mybir.dt.bfloat16
    i32 = mybir.dt.int32
    AF = mybir.ActivationFunctionType
    OP = mybir.AluOpType

    tok_total = ntiles * TILE

    nc = bacc.Bacc("TRN2", target_bir_lowering=False, debug=False,
                   enable_asserts=False, num_devices=NCORES)

    x_d = nc.dram_tensor("x", [ntiles, 128, NSUB * DP], f32,
                         kind="ExternalInput").ap()
    y_d = nc.dram_tensor("y", [ntiles, 128, NSUB * D], f32,
                         kind="ExternalOutput").ap()
    wd = []
    for i in range(DEPTH):
        wd.append({
            "wqk": nc.dram_tensor(f"wqk{i}", [D, 384], bf16, kind="ExternalInput").ap(),
            "bqk": nc.dram_tensor(f"bqk{i}", [128, 4], f32, kind="ExternalInput").ap(),
            "wv": nc.dram_tensor(f"wv{i}", [D, D], bf16, kind="ExternalInput").ap(),
            "bv": nc.dram_tensor(f"bv{i}", [128, 2], f32, kind="ExternalInput").ap(),
            "wp": nc.dram_tensor(f"wp{i}", [D, D], bf16, kind="ExternalInput").ap(),
            "bp": nc.dram_tensor(f"bp{i}", [128, D], f32, kind="ExternalInput").ap(),
            "w1": nc.dram_tensor(f"w1{i}", [D, DFF], bf16, kind="ExternalInput").ap(),
            "b1": nc.dram_tensor(f"b1{i}", [128, 6], f32, kind="ExternalInput").ap(),
            "w2": nc.dram_tensor(f"w2{i}", [DFF, D], bf16, kind="ExternalInput").ap(),
            "b2": nc.dram_tensor(f"b2{i}", [128, D], f32, kind="ExternalInput").ap(),
        })

    with tile.TileContext(nc) as tc, ExitStack() as ctx:
        consts = ctx.enter_context(tc.tile_pool(name="consts", bufs=1))
        xpool = ctx.enter_context(tc.tile_pool(name="xpool", bufs=10))
        lnpool = ctx.enter_context(tc.tile_pool(name="lnpool", bufs=6))
        fmpool = ctx.enter_context(tc.tile_pool(name="fmpool", bufs=10))
        qkpool = ctx.enter_context(tc.tile_pool(name="qkpool", bufs=8))
        apool = ctx.enter_context(tc.tile_pool(name="apool", bufs=3))
        pkpool = ctx.enter_context(tc.tile_pool(name="pkpool", bufs=7))
        ofpool = ctx.enter_context(tc.tile_pool(name="ofpool", bufs=8))
        hpool = ctx.enter_context(tc.tile_pool(name="hpool", bufs=2))
        stpool = ctx.enter_context(tc.tile_pool(name="stpool", bufs=9))
        ppsc = ctx.enter_context(tc.tile_pool(name="ppsc", bufs=1, space="PSUM"))
        ppm = ctx.enter_context(tc.tile_pool(name="ppm", bufs=4, space="PSUM"))

        # persistent softmax buffers: off-diagonal blocks stay 0 forever
        NPBUF = 4
        p_bufs = []
        for pb_i in range(NPBUF):
            pb = consts.tile([128, 2, HEADS, 128], bf16, name=f"pbuf{pb_i}")
            nc.vector.memset(pb, 0.0)
            p_bufs.append(pb)

        # --- load weights into SBUF once ---
        W = []
        for i in range(DEPTH):
            d = wd[i]
            sb = {}
            sb["wqk0"] = consts.tile([128, 384], bf16, name=f"wqk0{i}")
            sb["wqk1"] = consts.tile([64, 384], bf16, name=f"wqk1{i}")
            nc.scalar.dma_start(out=sb["wqk0"], in_=d["wqk"][0:128])
            nc.scalar.dma_start(out=sb["wqk1"], in_=d["wqk"][128:192])
            sb["wv0"] = consts.tile([128, D], bf16, name=f"wv0{i}")
            sb["wv1"] = consts.tile([64, D], bf16, name=f"wv1{i}")
            nc.scalar.dma_start(out=sb["wv0"], in_=d["wv"][0:128])
            nc.scalar.dma_start(out=sb["wv1"], in_=d["wv"][128:192])
            sb["wp0"] = consts.tile([128, D], bf16, name=f"wp0{i}")
            sb["wp1"] = consts.tile([64, D], bf16, name=f"wp1{i}")
            nc.scalar.dma_start(out=sb["wp0"], in_=d["wp"][0:128])
            nc.scalar.dma_start(out=sb["wp1"], in_=d["wp"][128:192])
            sb["w10"] = consts.tile([128, DFF], bf16, name=f"w10{i}")
            sb["w11"] = consts.tile([64, DFF], bf16, name=f"w11{i}")
            nc.scalar.dma_start(out=sb["w10"], in_=d["w1"][0:128])
            nc.scalar.dma_start(out=sb["w11"], in_=d["w1"][128:192])
            sb["w2m"] = consts.tile([128, 6, D], bf16, name=f"w2m{i}")
            nc.scalar.dma_start(
                out=sb["w2m"],
                in_=d["w2"].rearrange("(m p) n -> p m n", p=128))
            for nm in ("bqk", "bv", "b1", "bp", "b2"):
                shp = {"bqk": [128, 4], "bv": [128, 2], "b1": [128, 6],
                       "bp": [128, D], "b2": [128, D]}[nm]
                sb[nm] = consts.tile(shp, f32, name=f"{nm}{i}")
                nc.scalar.dma_start(out=sb[nm], in_=d[nm])
            W.append(sb)

        pair_ctr = [0]
        MAGIC = 0x5F3759DF
        # CoreSim lacks Gelu_apprx_tanh; substitute Tanh for sim-only runs.
        GELU_FUNC = (AF.Tanh if os.environ.get("K_SIM_GELU_TANH") == "1"
                     else AF.Gelu_apprx_tanh)

        def layernorm_fm(x_t, tag):
            """LN on token-major x_t -> feature-major bf16 via DMA transpose.
            Returns fmA [128,4,128] (feats 0:128, cols=tokens) and fmA2
            (feats 128:256; partitions 64:128 are pad)."""
            mv = stpool.tile([128, 4, 6], f32, tag="mv", name="mv")
            mv2 = stpool.tile([128, 4, 2], f32, tag="mv2", name="mv2")
            for s in range(NSUB):
                nc.vector.bn_stats(mv[:, s], x_t[:, s, 0:D])
                nc.vector.bn_aggr(mv2[:, s], mv[:, s])
            var = mv2[:, :, 1]                       # [128, 4] stride 2
            t_i = stpool.tile([128, 4], i32, tag="ti", name="t_i")
            y0 = stpool.tile([128, 4], f32, tag="y0", name="y0")
            zz = stpool.tile([128, 4], f32, tag="zz", name="zz")
            r4 = stpool.tile([128, 4], f32, tag="r4", name="r4")
            nc.vector.tensor_scalar(
                out=t_i, in0=var.bitcast(i32), scalar1=1, scalar2=None,
                op0=OP.logical_shift_right)
            nc.vector.tensor_scalar(
                out=y0.bitcast(i32), in0=t_i, scalar1=MAGIC, scalar2=-1,
                op0=OP.subtract, op1=OP.mult)
            nc.vector.scalar_tensor_tensor(
                out=zz, in0=var, scalar=1e-5, in1=y0,
                op0=OP.add, op1=OP.mult)              # (var+eps)*y0
            nc.vector.tensor_tensor(out=zz, in0=zz, in1=y0, op=OP.mult)
            nc.vector.tensor_scalar(
                out=zz, in0=zz, scalar1=-0.5, scalar2=1.5,
                op0=OP.mult, op1=OP.add)              # 1.5 - 0.5 v y0^2
            nc.vector.tensor_tensor(out=r4, in0=zz, in1=y0, op=OP.mult)

            xn = lnpool.tile([128, 2, NSUB, 128], bf16, tag="xn",
                             name=f"xn{tag}")
            for s in range(NSUB):
                nc.vector.tensor_scalar(
                    out=xn[:, :, s], in0=x_t[:, s].rearrange("p (c f) -> p c f", c=2),
                    scalar1=mv2[:, s, 0:1], scalar2=r4[:, s:s + 1],
                    op0=OP.subtract, op1=OP.mult)
            fm2 = fmpool.tile([128, 2, NSUB, 128], bf16, tag="fm",
                              name=f"fm{tag}")
            nc.sync.dma_start_transpose(out=fm2, in_=xn)
            return fm2[:, 0], fm2[:, 1]

        def phase_a(sb, fmA, fmA2):
            """qkv + v from feature-major LN output. Returns (qkA, qkB, v_tm)."""
            fmAf = fmA.rearrange("p a b -> p (a b)")
            fmA2f = fmA2.rearrange("p a b -> p (a b)")
            # qkv: m-chunks 0,1 are 128-wide (heads 0-3 q|k), 2,3 are 64-wide
            qkA = qkpool.tile([128, 2, TILE], bf16, tag="qkA", name="qkA")
            qkB = qkpool.tile([64, 2, TILE], bf16, tag="qkB", name="qkB")
            psq = []
            for m in range(2):
                ps = ppm.tile([128, TILE], f32, tag="med", name=f"psqA{m}")
                nc.tensor.matmul(ps, sb["wqk0"][:, ts(m, 128)], fmAf,
                                 start=True, stop=False)
                nc.tensor.matmul(ps, sb["wqk1"][:, ts(m, 128)],
                                 fmA2f[0:64], start=False, stop=True)
                psq.append(ps)
            for m in range(2):
                ps = ppm.tile([64, TILE], f32, tag="med", name=f"psqB{m}")
                nc.tensor.matmul(ps, sb["wqk0"][:, ds(256 + m * 64, 64)],
                                 fmAf, start=True, stop=False)
                nc.tensor.matmul(ps, sb["wqk1"][:, ds(256 + m * 64, 64)],
                                 fmA2f[0:64], start=False, stop=True)
                psq.append(ps)
            for m in range(2):
                if biases_zero:
                    nc.scalar.activation(qkA[:, m], psq[m], AF.Copy)
                    nc.vector.tensor_copy(qkB[:, m], psq[2 + m])
                else:
                    nc.scalar.activation(qkA[:, m], psq[m], AF.Identity,
                                         bias=sb["bqk"][:, m:m + 1])
                    nc.scalar.activation(qkB[:, m], psq[2 + m], AF.Identity,
                                         bias=sb["bqk"][0:64, 2 + m:3 + m])
            # v (token-major out), pairs of subs per PSUM tile
            v_tm = qkpool.tile([128, NSUB, D], bf16, tag="vtm", name="v_tm")
            for sp in range(2):
                psv = ppm.tile([128, 2, 256], f32, tag="med", name="psv")
                for j in range(2):
                    s = sp * 2 + j
                    nc.tensor.matmul(psv[:, j, 0:D], fmA[:, s], sb["wv0"],
                                     start=True, stop=False)
                    nc.tensor.matmul(psv[:, j, 0:D], fmA2[0:64, s], sb["wv1"],
                                     start=False, stop=True)
                if biases_zero:
                    nc.scalar.activation(v_tm[:, ds(sp * 2, 2)],
                                         psv[:, :, 0:D], AF.Copy)
                else:
                    for j in range(2):
                        nc.scalar.activation(
                            v_tm[:, sp * 2 + j], psv[:, j, 0:D], AF.Identity,
                            bias=sb["bv"][:, 0:1])
            return qkA, qkB, v_tm

        def phase_b_soft(sb, qkA, qkB, sp):
            """scores + softmax + P^T for one sub-pair; returns pkm2 tile."""
            P2 = p_bufs[pair_ctr[0] % NPBUF]
            pair_ctr[0] += 1
            pkm2 = pkpool.tile([128, 2, HEADS, 128], bf16, tag="pkm",
                               name="pkm")
            for j in range(2):
                s = sp * 2 + j
                cols = ts(s, 128)
                sc = ppsc.tile([128, 4, 512], f32, tag="sc", name="sc")
                for h in range(HEADS):
                    if h < 4:
                        qs = qkA[ts(h, 32), 0, cols]
                        ks = qkA[ts(h, 32), 1, cols]
                    else:
                        qs = qkB[ts(h - 4, 32), 0, cols]
                        ks = qkB[ts(h - 4, 32), 1, cols]
                    out = sc[:, h % 4, ds((h // 4) * 128, 128)]
                    nc.tensor.matmul(out, qs, ks,
                                     start=True, stop=True,
                                     tile_position=(32 * (h % 4), 0))
                E = apool.tile([128, HEADS, 128], bf16, tag="E", name="E")
                sums = stpool.tile([128, HEADS], f32, tag="sm", name="sums")
                rsum = stpool.tile([128, HEADS], f32, tag="rs", name="rsum")
                nc.scalar.activation(E[:, 0:4], sc[:, :, 0:128], AF.Exp)
                nc.scalar.activation(E[:, 4:6], sc[:, 0:2, 128:256], AF.Exp)
                nc.vector.reduce_sum(sums[0:64], E[0:64, :, 0:64],
                                     axis=mybir.AxisListType.X)
                nc.vector.reduce_sum(sums[64:128], E[64:128, :, 64:128],
                                     axis=mybir.AxisListType.X)
                nc.vector.reciprocal(rsum, sums)
                P = P2[:, j]
                for half in range(2):
                    hs = ds(half * 64, 64)
                    rs_half = rsum[ds(half * 64, 64)]
                    rsum_b = bass.AP(tensor=rs_half.tensor,
                                     offset=rs_half.offset,
                                     ap=[*rs_half.ap, [0, 64]])
                    eng = nc.vector if half == 0 else nc.gpsimd
                    eng.tensor_tensor(
                        out=P[hs, :, hs], in0=E[hs, :, hs],
                        in1=rsum_b, op=OP.mult)
            nc.sync.dma_start_transpose(out=pkm2, in_=P2)
            return pkm2

        def phase_b_out(sb, v_tm, pkm2, ofmA, ofmB, sp):
            """attention O matmuls + feature-major output copies."""
            oPp = ppm.tile([128, 2, 256], f32, tag="med", name="oPp")
            for j in range(2):
                s = sp * 2 + j
                for h in range(HEADS):
                    if h < 4:
                        out = oPp[ts(h, 32), j, 0:128]
                        colpos = h * 32
                    else:
                        out = oPp[ts(h - 4, 32), j, 128:256]
                        colpos = (h - 4) * 32
                    nc.tensor.matmul(out, v_tm[:, s, ts(h, 32)],
                                     pkm2[:, j, h], start=True, stop=True,
                                     tile_position=(0, colpos))
            if biases_zero:
                nc.vector.tensor_copy(
                    ofmA.rearrange("p (a b) -> p a b", a=NSUB)[:, ds(sp * 2, 2)],
                    oPp[:, :, 0:128])
                nc.vector.tensor_copy(
                    ofmB.rearrange("p (a b) -> p a b", a=NSUB)[:, ds(sp * 2, 2)],
                    oPp[0:64, :, 128:256])
            else:
                c0 = sp * 256
                for j in range(2):
                    nc.scalar.activation(
                        ofmA[:, ds(c0 + j * 128, 128)], oPp[:, j, 0:128],
                        AF.Identity, bias=sb["bv"][:, 0:1])
                    nc.scalar.activation(
                        ofmB[:, ds(c0 + j * 128, 128)],
                        oPp[0:64, j, 128:256],
                        AF.Identity, bias=sb["bv"][0:64, 1:2])

        def phase_c_proj(sb, x_t, ofmA, ofmB):
            """proj + residual."""
            for sp in range(2):
                psp = ppm.tile([128, 2, 256], f32, tag="med", name="psp")
                for j in range(2):
                    s = sp * 2 + j
                    nc.tensor.matmul(psp[:, j, 0:D], ofmA[:, ts(s, 128)],
                                     sb["wp0"], start=True, stop=False)
                    nc.tensor.matmul(psp[:, j, 0:D], ofmB[:, ts(s, 128)],
                                     sb["wp1"], start=False, stop=True)
                nc.vector.tensor_add(x_t[:, ds(sp * 2, 2), 0:D],
                                     x_t[:, ds(sp * 2, 2), 0:D],
                                     psp[:, :, 0:D])
                if not biases_zero:
                    for j in range(2):
                        nc.vector.tensor_add(x_t[:, sp * 2 + j, 0:D],
                                             x_t[:, sp * 2 + j, 0:D], sb["bp"])
        def phase_c_mlp(sb, x_t, ynA, ynA2):
            """MLP + residual."""
            ynAf = ynA.rearrange("p a b -> p (a b)")
            ynA2f = ynA2.rearrange("p a b -> p (a b)")
            hfm = hpool.tile([128, 6, TILE], bf16, tag="hfm", name="hfm")
            for m in range(6):
                psf = ppm.tile([128, TILE], f32, tag="med", name="psf1")
                nc.tensor.matmul(psf, sb["w10"][:, ts(m, 128)],
                                 ynAf, start=True, stop=False)
                nc.tensor.matmul(psf, sb["w11"][:, ts(m, 128)],
                                 ynA2f[0:64], start=False, stop=True)
                if biases_zero:
                    nc.scalar.activation(hfm[:, m], psf, GELU_FUNC)
                else:
                    nc.scalar.activation(hfm[:, m], psf, GELU_FUNC,
                                         bias=sb["b1"][:, m:m + 1])
            for sp in range(2):
                psf2 = ppm.tile([128, 2, 256], f32, tag="med", name="psf2")
                for j in range(2):
                    s = sp * 2 + j
                    for m in range(6):
                        nc.tensor.matmul(psf2[:, j, 0:D],
                                         hfm[:, m, ts(s, 128)],
                                         sb["w2m"][:, m],
                                         start=(m == 0), stop=(m == 5))
                nc.vector.tensor_add(x_t[:, ds(sp * 2, 2), 0:D],
                                     x_t[:, ds(sp * 2, 2), 0:D],
                                     psf2[:, :, 0:D])
                if not biases_zero:
                    for j in range(2):
                        nc.vector.tensor_add(x_t[:, sp * 2 + j, 0:D],
                                             x_t[:, sp * 2 + j, 0:D], sb["b2"])

        ngroups = (ntiles + GROUP - 1) // GROUP
        HOIST = 0

        def load_x(it):
            x_t = xpool.tile([128, NSUB, DP], f32, tag="x", name="x_t")
            nc.sync.dma_start(
                out=x_t,
                in_=x_d[it].rearrange("p (s f) -> p s f", s=NSUB))
            return x_t

        carry_x = {}
        carry_fms = {}
        for g in range(ngroups):
            tiles = [g * GROUP + t for t in range(GROUP)
                     if g * GROUP + t < ntiles]
            xts = dict(carry_x)
            carry_x = {}
            for it in tiles:
                if it not in xts:
                    xts[it] = load_x(it)
            for li in range(DEPTH):
                sb = W[li]
                fms = {}
                for it in tiles:
                    if li == 0 and it in carry_fms:
                        fms[it] = carry_fms.pop(it)
                    else:
                        fms[it] = layernorm_fm(xts[it], "1")
                qk = {}
                for it in tiles:
                    qk[it] = phase_a(sb, *fms[it])
                of = {}
                for it in tiles:
                    of[it] = (ofpool.tile([128, TILE], bf16, tag="ofA",
                                          name="ofmA"),
                              ofpool.tile([64, TILE], bf16, tag="ofB",
                                          name="ofmB"))
                chains = [(it, sp) for it in tiles for sp in range(2)]
                SKEW = 5
                pk = {}
                for ci in range(len(chains) + SKEW):
                    if ci < len(chains):
                        it, sp = chains[ci]
                        qkA, qkB, _ = qk[it]
                        pk[ci] = phase_b_soft(sb, qkA, qkB, sp)
                    if ci >= SKEW:
                        it, sp = chains[ci - SKEW]
                        _, _, v_tm = qk[it]
                        phase_b_out(sb, v_tm, pk.pop(ci - SKEW),
                                    of[it][0], of[it][1], sp)
                for it in tiles:
                    ofmA, ofmB = of[it]
                    phase_c_proj(sb, xts[it], ofmA, ofmB)
                yns = {}
                for it in tiles:
                    yns[it] = layernorm_fm(xts[it], "2")
                if li == DEPTH - 1 and g + 1 < ngroups:
                    for nt in range(HOIST):
                        it2 = (g + 1) * GROUP + nt
                        if it2 < ntiles:
                            carry_x[it2] = load_x(it2)
                    for it2 in list(carry_x):
                        carry_fms[it2] = layernorm_fm(carry_x[it2], "1")
                for it in tiles:
                    phase_c_mlp(sb, xts[it], *yns[it])
            for it in tiles:
                nc.sync.dma_start(
                    out=y_d[it].rearrange("p (s f) -> p s f", s=NSUB),
                    in_=xts[it][:, :, 0:D])

    nc.compile()
    _COMPILED[key] = nc
    return nc


def _ensure_ntff_hook():
    import sys, types
    if "antenv.axon_hooks" in sys.modules:
        return True
    try:
        mod = types.ModuleType("antenv.axon_hooks")
        state = {}
        mod.set_axon_ntff_profile_hook = lambda h: state.__setitem__("h", h)
        mod.get_axon_ntff_profile_hook = lambda: state.get("h")
        sys.modules["antenv.axon_hooks"] = mod
        import antenv
        antenv.axon_hooks = mod
        from trn_agent_boot.trn_boot import _ntff_profile_via_ctypes
        mod.set_axon_ntff_profile_hook(
            _ntff_profile_via_ctypes("/opt/axon/libaxon_pjrt.so"))
        return True
    except Exception as e:  # pragma: no cover
        print(f"NTFF hook shim failed: {e}")
        return False


def _run(inputs, trace=False):
    """Shard, execute on 8 cores, gather. Returns (y_full, exec_time_ns)."""
    from concourse.bass_utils import run_bass_kernel_spmd

    if trace:
        trace = _ensure_ntff_hook()

    layers = _fold_weights(inputs)
    bz = all(
        not np.any(np.asarray(d[k], np.float32))
        for d in layers for k in d if k.startswith(("bqk", "bv", "bp", "b1", "b2")))
    nc = _build_nc(biases_zero=bz)

    x = np.asarray(inputs["x"], np.float32)
    pos = np.asarray(inputs["pos"], np.float32)
    w = int(np.asarray(inputs["w"]))
    order = _scanline_order(pos, w)
    x_ord = np.take_along_axis(x, order[..., None], axis=1)
    # device layout: [NTILES, 128 (token-in-sub), NSUB, DP]
    sw = np.zeros((NCORES, NTILES, 128, NSUB, DP), np.float32)
    sw[..., 0:D] = x_ord.reshape(NCORES, NTILES, NSUB, 128, D).transpose(
        0, 1, 3, 2, 4)
    shards = sw.reshape(NCORES, NTILES, 128, NSUB * DP)

    wmap = {}
    for d in layers:
        wmap.update({k: np.ascontiguousarray(v) for k, v in d.items()})

    in_maps = [{"x": shards[c], **wmap} for c in range(NCORES)]
    res = run_bass_kernel_spmd(nc, in_maps, core_ids=list(range(NCORES)),
                               trace=trace)
    y_ord = np.stack([res.results[c]["y"] for c in range(NCORES)])
    y_ord = y_ord.reshape(NCORES, NTILES, 128, NSUB, D).transpose(
        0, 1, 3, 2, 4)
    y_ord = y_ord.reshape(B, N, D)
    y = np.empty_like(y_ord)
    np.put_along_axis(y, order[..., None], y_ord, axis=1)
    return y.astype(np.float32), res.exec_time_ns


def kernel(**inputs):
    y, _ = _run(inputs, trace=False)
    return y


# revision 22
# speedup vs baseline: 1.0162x; 1.0015x over previous
"""Trainium2 Bass kernel for nn_BasicLayer (sparse cluster attention, 2 layers).

v2 rewrite of the staged baseline. Same host-side strategy (scanline gather,
8 cores x 8192 tokens, folded weights, token-major fp32 residual, bf16 matmul
operands) with an on-device restructure aimed at engine balance and PE
density:

- All layout flips (LN token-major -> feature-major, P -> P^T) go through the
  DMA xbar transpose engine instead of PE transpose + PSUM copy.
- LN: one batched bn_stats pair, rsqrt via fast-inverse-sqrt bit trick +
  1 Newton step (DVE only, no sqrt table), normalize via dual-scalar
  tensor_scalar producing bf16 directly.
- Scores matmuls read per-head q/k slices in place via tile_position row
  packing (no per-head copies).
- Softmax: exp -> scratch E; P = E * (1/rowsum) only on the diagonal
  64x64 blocks into persistent zeroed P buffers (gpsimd); P^T via DMA
  transpose feeds the O matmuls.
- Supertiles processed in groups of 4 with phase-major ordering per layer so
  the scalar engine's activation-table switches (Exp <-> Gelu) amortize
  across the group.
"""

import os
import numpy as np
import ml_dtypes

# ---- problem constants (hardcoded per contract) ----
B, N, D = 4, 16384, 192
DP = 256
HEADS, DH, CLM = 6, 32, 64
GRID_W = 128
DEPTH = 2
NCORES = 8
T = (B * N) // NCORES                # 8192 tokens per core
SUB = 128
NSUB = 4
TILE = SUB * NSUB                    # 512-token supertile
NTILES = T // TILE                   # 16
GROUP = 8                            # supertiles per phase group
DFF = 768

_COMPILED = {}


def _scanline_order(pos, w):
    ix = np.floor(pos[..., 0]).astype(np.int64)
    iy = np.floor(pos[..., 1]).astype(np.int64)
    key = iy * w + np.where(iy % 2 == 1, w - 1 - ix, ix)
    return np.argsort(key, axis=1, kind="stable")


def _fold_weights(inputs):
    """Fold LN affine + biases into matmul weights (same layout as v1)."""
    bf16 = ml_dtypes.bfloat16
    scale = DH ** -0.5
    layers = []
    for i in range(DEPTH):
        g1 = np.asarray(inputs["ln1_g"][i], np.float64)
        b1 = np.asarray(inputs["ln1_b"][i], np.float64)
        Wqkv = np.asarray(inputs["w_qkv"][i], np.float64)
        bqkv = np.asarray(inputs["b_qkv"][i], np.float64)
        w_eff = g1[:, None] * Wqkv
        b_eff = b1 @ Wqkv + bqkv
        wq = w_eff[:, 0:D] * scale
        bq = b_eff[0:D] * scale
        wk = w_eff[:, D:2 * D]
        bk = b_eff[D:2 * D]
        wv = w_eff[:, 2 * D:3 * D]
        bv = b_eff[2 * D:3 * D]
        wqk = np.concatenate(
            [wq[:, :128], wk[:, :128], wq[:, 128:], wk[:, 128:]], axis=1)
        pad64 = np.zeros(64)
        bqk = np.stack(
            [bq[:128], bk[:128],
             np.concatenate([bq[128:], pad64]),
             np.concatenate([bk[128:], pad64])], axis=1)
        wp = np.asarray(inputs["w_proj"][i], np.float64)
        bp = np.asarray(inputs["b_proj"][i], np.float64)
        g2 = np.asarray(inputs["ln2_g"][i], np.float64)
        b2 = np.asarray(inputs["ln2_b"][i], np.float64)
        W1 = np.asarray(inputs["w_fc1"][i], np.float64)
        w1_eff = g2[:, None] * W1
        b1_eff = b2 @ W1 + np.asarray(inputs["b_fc1"][i], np.float64)
        W2 = np.asarray(inputs["w_fc2"][i], np.float64)
        bfc2 = np.asarray(inputs["b_fc2"][i], np.float64)
        bv_t = np.stack(
            [bv[:128], np.concatenate([bv[128:], np.zeros(64)])], axis=1)
        layers.append({
            f"wqk{i}": wqk.astype(bf16),
            f"bqk{i}": bqk.astype(np.float32),
            f"wv{i}": wv.astype(bf16),
            f"bv{i}": bv_t.astype(np.float32),
            f"wp{i}": wp.astype(bf16),
            f"bp{i}": np.tile(bp.astype(np.float32), (128, 1)),
            f"w1{i}": w1_eff.astype(bf16),
            f"b1{i}": b1_eff.reshape(6, 128).T.copy().astype(np.float32),
            f"w2{i}": W2.astype(bf16),
            f"b2{i}": np.tile(bfc2.astype(np.float32), (128, 1)),
        })
    return layers


def _build_nc(biases_zero=True, ntiles=NTILES):
    key = ("nc", biases_zero, ntiles)
    if key in _COMPILED:
        return _COMPILED[key]

    from contextlib import ExitStack
    import concourse.bass as bass
    import concourse.tile as tile
    from concourse import bacc, mybir
    from concourse.bass import ts, ds

    f32 = mybir.dt.float32
    bf16 = mybir.dt.bfloat16
    i32 = mybir.dt.int32
    AF = mybir.ActivationFunctionType
    OP = mybir.AluOpType

    tok_total = ntiles * TILE

    nc = bacc.Bacc("TRN2", target_bir_lowering=False, debug=False,
                   enable_asserts=False, num_devices=NCORES)

    x_d = nc.dram_tensor("x", [ntiles, 128, NSUB * DP], f32,
                         kind="ExternalInput").ap()
    y_d = nc.dram_tensor("y", [ntiles, 128, NSUB * D], f32,
                         kind="ExternalOutput").ap()
    wd = []
    for i in range(DEPTH):
        wd.append({
            "wqk": nc.dram_tensor(f"wqk{i}", [D, 384], bf16, kind="ExternalInput").ap(),
            "bqk": nc.dram_tensor(f"bqk{i}", [128, 4], f32, kind="ExternalInput").ap(),
            "wv": nc.dram_tensor(f"wv{i}", [D, D], bf16, kind="ExternalInput").ap(),
            "bv": nc.dram_tensor(f"bv{i}", [128, 2], f32, kind="ExternalInput").ap(),
            "wp": nc.dram_tensor(f"wp{i}", [D, D], bf16, kind="ExternalInput").ap(),
            "bp": nc.dram_tensor(f"bp{i}", [128, D], f32, kind="ExternalInput").ap(),
            "w1": nc.dram_tensor(f"w1{i}", [D, DFF], bf16, kind="ExternalInput").ap(),
            "b1": nc.dram_tensor(f"b1{i}", [128, 6], f32, kind="ExternalInput").ap(),
            "w2": nc.dram_tensor(f"w2{i}", [DFF, D], bf16, kind="ExternalInput").ap(),
            "b2": nc.dram_tensor(f"b2{i}", [128, D], f32, kind="ExternalInput").ap(),
        })

    with tile.TileContext(nc) as tc, ExitStack() as ctx:
        consts = ctx.enter_context(tc.tile_pool(name="consts", bufs=1))
        xpool = ctx.enter_context(tc.tile_pool(name="xpool", bufs=9))
        lnpool = ctx.enter_context(tc.tile_pool(name="lnpool", bufs=6))
        fmpool = ctx.enter_context(tc.tile_pool(name="fmpool", bufs=9))
        qkpool = ctx.enter_context(tc.tile_pool(name="qkpool", bufs=8))
        apool = ctx.enter_context(tc.tile_pool(name="apool", bufs=3))
        pkpool = ctx.enter_context(tc.tile_pool(name="pkpool", bufs=7))
        ofpool = ctx.enter_context(tc.tile_pool(name="ofpool", bufs=8))
        hpool = ctx.enter_context(tc.tile_pool(name="hpool", bufs=2))
        stpool = ctx.enter_context(tc.tile_pool(name="stpool", bufs=9))
        ppsc = ctx.enter_context(tc.tile_pool(name="ppsc", bufs=1, space="PSUM"))
        ppm = ctx.enter_context(tc.tile_pool(name="ppm", bufs=4, space="PSUM"))

        # persistent softmax buffers: off-diagonal blocks stay 0 forever
        NPBUF = 6
        p_bufs = []
        for pb_i in range(NPBUF):
            pb = consts.tile([128, 2, HEADS, 128], bf16, name=f"pbuf{pb_i}")
            nc.vector.memset(pb, 0.0)
            p_bufs.append(pb)

        # --- load weights into SBUF once ---
        W = []
        for i in range(DEPTH):
            d = wd[i]
            sb = {}
            sb["wqk0"] = consts.tile([128, 384], bf16, name=f"wqk0{i}")
            sb["wqk1"] = consts.tile([64, 384], bf16, name=f"wqk1{i}")
            nc.scalar.dma_start(out=sb["wqk0"], in_=d["wqk"][0:128])
            nc.scalar.dma_start(out=sb["wqk1"], in_=d["wqk"][128:192])
            sb["wv0"] = consts.tile([128, D], bf16, name=f"wv0{i}")
            sb["wv1"] = consts.tile([64, D], bf16, name=f"wv1{i}")
            nc.scalar.dma_start(out=sb["wv0"], in_=d["wv"][0:128])
            nc.scalar.dma_start(out=sb["wv1"], in_=d["wv"][128:192])
            sb["wp0"] = consts.tile([128, D], bf16, name=f"wp0{i}")
            sb["wp1"] = consts.tile([64, D], bf16, name=f"wp1{i}")
            nc.scalar.dma_start(out=sb["wp0"], in_=d["wp"][0:128])
            nc.scalar.dma_start(out=sb["wp1"], in_=d["wp"][128:192])
            sb["w10"] = consts.tile([128, DFF], bf16, name=f"w10{i}")
            sb["w11"] = consts.tile([64, DFF], bf16, name=f"w11{i}")
            nc.scalar.dma_start(out=sb["w10"], in_=d["w1"][0:128])
            nc.scalar.dma_start(out=sb["w11"], in_=d["w1"][128:192])
            sb["w2m"] = consts.tile([128, 6, D], bf16, name=f"w2m{i}")
            nc.scalar.dma_start(
                out=sb["w2m"],
                in_=d["w2"].rearrange("(m p) n -> p m n", p=128))
            for nm in ("bqk", "bv", "b1", "bp", "b2"):
                shp = {"bqk": [128, 4], "bv": [128, 2], "b1": [128, 6],
                       "bp": [128, D], "b2": [128, D]}[nm]
                sb[nm] = consts.tile(shp, f32, name=f"{nm}{i}")
                nc.scalar.dma_start(out=sb[nm], in_=d[nm])
            W.append(sb)

        pair_ctr = [0]
        MAGIC = 0x5F3759DF
        # CoreSim lacks Gelu_apprx_tanh; substitute Tanh for sim-only runs.
        GELU_FUNC = (AF.Tanh if os.environ.get("K_SIM_GELU_TANH") == "1"
                     else AF.Gelu_apprx_tanh)

        def layernorm_fm(x_t, tag):
            """LN on token-major x_t -> feature-major bf16 via DMA transpose.
            Returns fmA [128,4,128] (feats 0:128, cols=tokens) and fmA2
            (feats 128:256; partitions 64:128 are pad)."""
            mv = stpool.tile([128, 4, 6], f32, tag="mv", name="mv")
            mv2 = stpool.tile([128, 4, 2], f32, tag="mv2", name="mv2")
            for s in range(NSUB):
                nc.vector.bn_stats(mv[:, s], x_t[:, s, 0:D])
                nc.vector.bn_aggr(mv2[:, s], mv[:, s])
            var = mv2[:, :, 1]                       # [128, 4] stride 2
            t_i = stpool.tile([128, 4], i32, tag="ti", name="t_i")
            y0 = stpool.tile([128, 4], f32, tag="y0", name="y0")
            zz = stpool.tile([128, 4], f32, tag="zz", name="zz")
            r4 = stpool.tile([128, 4], f32, tag="r4", name="r4")
            nc.vector.tensor_scalar(
                out=t_i, in0=var.bitcast(i32), scalar1=1, scalar2=None,
                op0=OP.logical_shift_right)
            nc.vector.tensor_scalar(
                out=y0.bitcast(i32), in0=t_i, scalar1=MAGIC, scalar2=-1,
                op0=OP.subtract, op1=OP.mult)
            nc.vector.scalar_tensor_tensor(
                out=zz, in0=var, scalar=1e-5, in1=y0,
                op0=OP.add, op1=OP.mult)              # (var+eps)*y0
            nc.vector.tensor_tensor(out=zz, in0=zz, in1=y0, op=OP.mult)
            nc.vector.tensor_scalar(
                out=zz, in0=zz, scalar1=-0.5, scalar2=1.5,
                op0=OP.mult, op1=OP.add)              # 1.5 - 0.5 v y0^2
            nc.vector.tensor_tensor(out=r4, in0=zz, in1=y0, op=OP.mult)

            xn = lnpool.tile([128, 2, NSUB, 128], bf16, tag="xn",
                             name=f"xn{tag}")
            for s in range(NSUB):
                nc.vector.tensor_scalar(
                    out=xn[:, :, s], in0=x_t[:, s].rearrange("p (c f) -> p c f", c=2),
                    scalar1=mv2[:, s, 0:1], scalar2=r4[:, s:s + 1],
                    op0=OP.subtract, op1=OP.mult)
            fm2 = fmpool.tile([128, 2, NSUB, 128], bf16, tag="fm",
                              name=f"fm{tag}")
            nc.sync.dma_start_transpose(out=fm2, in_=xn)
            return fm2[:, 0], fm2[:, 1]

        def phase_a(sb, fmA, fmA2):
            """qkv + v from feature-major LN output. Returns (qkA, qkB, v_tm)."""
            fmAf = fmA.rearrange("p a b -> p (a b)")
            fmA2f = fmA2.rearrange("p a b -> p (a b)")
            # qkv: m-chunks 0,1 are 128-wide (heads 0-3 q|k), 2,3 are 64-wide
            qkA = qkpool.tile([128, 2, TILE], bf16, tag="qkA", name="qkA")
            qkB = qkpool.tile([64, 2, TILE], bf16, tag="qkB", name="qkB")
            psq = []
            for m in range(2):
                ps = ppm.tile([128, TILE], f32, tag="med", name=f"psqA{m}")
                nc.tensor.matmul(ps, sb["wqk0"][:, ts(m, 128)], fmAf,
                                 start=True, stop=False)
                nc.tensor.matmul(ps, sb["wqk1"][:, ts(m, 128)],
                                 fmA2f[0:64], start=False, stop=True)
                psq.append(ps)
            for m in range(2):
                ps = ppm.tile([64, TILE], f32, tag="med", name=f"psqB{m}")
                nc.tensor.matmul(ps, sb["wqk0"][:, ds(256 + m * 64, 64)],
                                 fmAf, start=True, stop=False)
                nc.tensor.matmul(ps, sb["wqk1"][:, ds(256 + m * 64, 64)],
                                 fmA2f[0:64], start=False, stop=True)
                psq.append(ps)
            for m in range(2):
                if biases_zero:
                    nc.scalar.activation(qkA[:, m], psq[m], AF.Copy)
                    nc.vector.tensor_copy(qkB[:, m], psq[2 + m])
                else:
                    nc.scalar.activation(qkA[:, m], psq[m], AF.Identity,
                                         bias=sb["bqk"][:, m:m + 1])
                    nc.scalar.activation(qkB[:, m], psq[2 + m], AF.Identity,
                                         bias=sb["bqk"][0:64, 2 + m:3 + m])
            # v (token-major out), pairs of subs per PSUM tile
            v_tm = qkpool.tile([128, NSUB, D], bf16, tag="vtm", name="v_tm")
            for sp in range(2):
                psv = ppm.tile([128, 2, 256], f32, tag="med", name="psv")
                for j in range(2):
                    s = sp * 2 + j
                    nc.tensor.matmul(psv[:, j, 0:D], fmA[:, s], sb["wv0"],
                                     start=True, stop=False)
                    nc.tensor.matmul(psv[:, j, 0:D], fmA2[0:64, s], sb["wv1"],
                                     start=False, stop=True)
                if biases_zero:
                    nc.scalar.activation(v_tm[:, ds(sp * 2, 2)],
                                         psv[:, :, 0:D], AF.Copy)
                else:
                    for j in range(2):
                        nc.scalar.activation(
                            v_tm[:, sp * 2 + j], psv[:, j, 0:D], AF.Identity,
                            bias=sb["bv"][:, 0:1])
            return qkA, qkB, v_tm

        def phase_b_soft(sb, qkA, qkB, sp):
            """scores + softmax + P^T for one sub-pair; returns pkm2 tile."""
            P2 = p_bufs[pair_ctr[0] % NPBUF]
            pair_ctr[0] += 1
            pkm2 = pkpool.tile([128, 2, HEADS, 128], bf16, tag="pkm",
                               name="pkm")
            for j in range(2):
                s = sp * 2 + j
                cols = ts(s, 128)
                sc = ppsc.tile([128, 4, 512], f32, tag="sc", name="sc")
                for h in range(HEADS):
                    if h < 4:
                        qs = qkA[ts(h, 32), 0, cols]
                        ks = qkA[ts(h, 32), 1, cols]
                    else:
                        qs = qkB[ts(h - 4, 32), 0, cols]
                        ks = qkB[ts(h - 4, 32), 1, cols]
                    out = sc[:, h % 4, ds((h // 4) * 128, 128)]
                    nc.tensor.matmul(out, qs, ks,
                                     start=True, stop=True,
                                     tile_position=(32 * (h % 4), 0))
                E = apool.tile([128, HEADS, 128], bf16, tag="E", name="E")
                sums = stpool.tile([128, HEADS], f32, tag="sm", name="sums")
                rsum = stpool.tile([128, HEADS], f32, tag="rs", name="rsum")
                nc.scalar.activation(E[:, 0:4], sc[:, :, 0:128], AF.Exp)
                nc.scalar.activation(E[:, 4:6], sc[:, 0:2, 128:256], AF.Exp)
                nc.vector.reduce_sum(sums[0:64], E[0:64, :, 0:64],
                                     axis=mybir.AxisListType.X)
                nc.vector.reduce_sum(sums[64:128], E[64:128, :, 64:128],
                                     axis=mybir.AxisListType.X)
                nc.vector.reciprocal(rsum, sums)
                P = P2[:, j]
                for half in range(2):
                    hs = ds(half * 64, 64)
                    rs_half = rsum[ds(half * 64, 64)]
                    rsum_b = bass.AP(tensor=rs_half.tensor,
                                     offset=rs_half.offset,
                                     ap=[*rs_half.ap, [0, 64]])
                    eng = nc.vector if half == 0 else nc.gpsimd
                    eng.tensor_tensor(
                        out=P[hs, :, hs], in0=E[hs, :, hs],
                        in1=rsum_b, op=OP.mult)
            nc.sync.dma_start_transpose(out=pkm2, in_=P2)
            return pkm2

        def phase_b_out(sb, v_tm, pkm2, ofmA, ofmB, sp):
            """attention O matmuls + feature-major output copies."""
            oPp = ppm.tile([128, 2, 256], f32, tag="med", name="oPp")
            for j in range(2):
                s = sp * 2 + j
                for h in range(HEADS):
                    if h < 4:
                        out = oPp[ts(h, 32), j, 0:128]
                        colpos = h * 32
                    else:
                        out = oPp[ts(h - 4, 32), j, 128:256]
                        colpos = (h - 4) * 32
                    nc.tensor.matmul(out, v_tm[:, s, ts(h, 32)],
                                     pkm2[:, j, h], start=True, stop=True,
                                     tile_position=(0, colpos))
            if biases_zero:
                nc.vector.tensor_copy(
                    ofmA.rearrange("p (a b) -> p a b", a=NSUB)[:, ds(sp * 2, 2)],
                    oPp[:, :, 0:128])
                nc.vector.tensor_copy(
                    ofmB.rearrange("p (a b) -> p a b", a=NSUB)[:, ds(sp * 2, 2)],
                    oPp[0:64, :, 128:256])
            else:
                c0 = sp * 256
                for j in range(2):
                    nc.scalar.activation(
                        ofmA[:, ds(c0 + j * 128, 128)], oPp[:, j, 0:128],
                        AF.Identity, bias=sb["bv"][:, 0:1])
                    nc.scalar.activation(
                        ofmB[:, ds(c0 + j * 128, 128)],
                        oPp[0:64, j, 128:256],
                        AF.Identity, bias=sb["bv"][0:64, 1:2])

        def phase_c_proj(sb, x_t, ofmA, ofmB):
            """proj + residual."""
            for sp in range(2):
                psp = ppm.tile([128, 2, 256], f32, tag="med", name="psp")
                for j in range(2):
                    s = sp * 2 + j
                    nc.tensor.matmul(psp[:, j, 0:D], ofmA[:, ts(s, 128)],
                                     sb["wp0"], start=True, stop=False)
                    nc.tensor.matmul(psp[:, j, 0:D], ofmB[:, ts(s, 128)],
                                     sb["wp1"], start=False, stop=True)
                nc.vector.tensor_add(x_t[:, ds(sp * 2, 2), 0:D],
                                     x_t[:, ds(sp * 2, 2), 0:D],
                                     psp[:, :, 0:D])
                if not biases_zero:
                    for j in range(2):
                        nc.vector.tensor_add(x_t[:, sp * 2 + j, 0:D],
                                             x_t[:, sp * 2 + j, 0:D], sb["bp"])
        def phase_c_mlp(sb, x_t, ynA, ynA2):
            """MLP + residual."""
            ynAf = ynA.rearrange("p a b -> p (a b)")
            ynA2f = ynA2.rearrange("p a b -> p (a b)")
            hfm = hpool.tile([128, 6, TILE], bf16, tag="hfm", name="hfm")
            for m in range(6):
                psf = ppm.tile([128, TILE], f32, tag="med", name="psf1")
                nc.tensor.matmul(psf, sb["w10"][:, ts(m, 128)],
                                 ynAf, start=True, stop=False)
                nc.tensor.matmul(psf, sb["w11"][:, ts(m, 128)],
                                 ynA2f[0:64], start=False, stop=True)
                if biases_zero:
                    nc.scalar.activation(hfm[:, m], psf, GELU_FUNC)
                else:
                    nc.scalar.activation(hfm[:, m], psf, GELU_FUNC,
                                         bias=sb["b1"][:, m:m + 1])
            for sp in range(2):
                psf2 = ppm.tile([128, 2, 256], f32, tag="med", name="psf2")
                for j in range(2):
                    s = sp * 2 + j
                    for m in range(6):
                        nc.tensor.matmul(psf2[:, j, 0:D],
                                         hfm[:, m, ts(s, 128)],
                                         sb["w2m"][:, m],
                                         start=(m == 0), stop=(m == 5))
                nc.vector.tensor_add(x_t[:, ds(sp * 2, 2), 0:D],
                                     x_t[:, ds(sp * 2, 2), 0:D],
                                     psf2[:, :, 0:D])
                if not biases_zero:
                    for j in range(2):
                        nc.vector.tensor_add(x_t[:, sp * 2 + j, 0:D],
                                             x_t[:, sp * 2 + j, 0:D], sb["b2"])

        ngroups = (ntiles + GROUP - 1) // GROUP
        HOIST = 0

        def load_x(it):
            x_t = xpool.tile([128, NSUB, DP], f32, tag="x", name="x_t")
            nc.sync.dma_start(
                out=x_t,
                in_=x_d[it].rearrange("p (s f) -> p s f", s=NSUB))
            return x_t

        carry_x = {}
        carry_fms = {}
        for g in range(ngroups):
            tiles = [g * GROUP + t for t in range(GROUP)
                     if g * GROUP + t < ntiles]
            xts = dict(carry_x)
            carry_x = {}
            for it in tiles:
                if it not in xts:
                    xts[it] = load_x(it)
            for li in range(DEPTH):
                sb = W[li]
                fms = {}
                for it in tiles:
                    if li == 0 and it in carry_fms:
                        fms[it] = carry_fms.pop(it)
                    else:
                        fms[it] = layernorm_fm(xts[it], "1")
                qk = {}
                for it in tiles:
                    qk[it] = phase_a(sb, *fms[it])
                of = {}
                for it in tiles:
                    of[it] = (ofpool.tile([128, TILE], bf16, tag="ofA",
                                          name="ofmA"),
                              ofpool.tile([64, TILE], bf16, tag="ofB",
                                          name="ofmB"))
                chains = [(it, sp) for it in tiles for sp in range(2)]
                SKEW = 5
                pk = {}
                for ci in range(len(chains) + SKEW):
                    if ci < len(chains):
                        it, sp = chains[ci]
                        qkA, qkB, _ = qk[it]
                        pk[ci] = phase_b_soft(sb, qkA, qkB, sp)
                    if ci >= SKEW:
                        it, sp = chains[ci - SKEW]
                        _, _, v_tm = qk[it]
                        phase_b_out(sb, v_tm, pk.pop(ci - SKEW),
                                    of[it][0], of[it][1], sp)
                for it in tiles:
                    ofmA, ofmB = of[it]
                    phase_c_proj(sb, xts[it], ofmA, ofmB)
                yns = {}
                for it in tiles:
                    yns[it] = layernorm_fm(xts[it], "2")
                if li == DEPTH - 1 and g + 1 < ngroups:
                    for nt in range(HOIST):
                        it2 = (g + 1) * GROUP + nt
                        if it2 < ntiles:
                            carry_x[it2] = load_x(it2)
                    for it2 in list(carry_x):
                        carry_fms[it2] = layernorm_fm(carry_x[it2], "1")
                for it in tiles:
                    phase_c_mlp(sb, xts[it], *yns[it])
            for it in tiles:
                nc.sync.dma_start(
                    out=y_d[it].rearrange("p (s f) -> p s f", s=NSUB),
                    in_=xts[it][:, :, 0:D])

    nc.compile()
    _COMPILED[key] = nc
    return nc


def _ensure_ntff_hook():
    import sys, types
    if "antenv.axon_hooks" in sys.modules:
        return True
    try:
        mod = types.ModuleType("antenv.axon_hooks")
        state = {}
        mod.set_axon_ntff_profile_hook = lambda h: state.__setitem__("h", h)
        mod.get_axon_ntff_profile_hook = lambda: state.get("h")
        sys.modules["antenv.axon_hooks"] = mod
        import antenv
        antenv.axon_hooks = mod
        from trn_agent_boot.trn_boot import _ntff_profile_via_ctypes
        mod.set_axon_ntff_profile_hook(
            _ntff_profile_via_ctypes("/opt/axon/libaxon_pjrt.so"))
        return True
    except Exception as e:  # pragma: no cover
        print(f"NTFF hook shim failed: {e}")
        return False


def _run(inputs, trace=False):
    """Shard, execute on 8 cores, gather. Returns (y_full, exec_time_ns)."""
    from concourse.bass_utils import run_bass_kernel_spmd

    if trace:
        trace = _ensure_ntff_hook()

    layers = _fold_weights(inputs)
    bz = all(
        not np.any(np.asarray(d[k], np.float32))
        for d in layers for k in d if k.startswith(("bqk", "bv", "bp", "b1", "b2")))
    nc = _build_nc(biases_zero=bz)

    x = np.asarray(inputs["x"], np.float32)
    pos = np.asarray(inputs["pos"], np.float32)
    w = int(np.asarray(inputs["w"]))
    order = _scanline_order(pos, w)
    x_ord = np.take_along_axis(x, order[..., None], axis=1)
    # device layout: [NTILES, 128 (token-in-sub), NSUB, DP]
    sw = np.zeros((NCORES, NTILES, 128, NSUB, DP), np.float32)
    sw[..., 0:D] = x_ord.reshape(NCORES, NTILES, NSUB, 128, D).transpose(
        0, 1, 3, 2, 4)
    shards = sw.reshape(NCORES, NTILES, 128, NSUB * DP)

    wmap = {}
    for d in layers:
        wmap.update({k: np.ascontiguousarray(v) for k, v in d.items()})

    in_maps = [{"x": shards[c], **wmap} for c in range(NCORES)]
    res = run_bass_kernel_spmd(nc, in_maps, core_ids=list(range(NCORES)),
                               trace=trace)
    y_ord = np.stack([res.results[c]["y"] for c in range(NCORES)])
    y_ord = y_ord.reshape(NCORES, NTILES, 128, NSUB, D).transpose(
        0, 1, 3, 2, 4)
    y_ord = y_ord.reshape(B, N, D)
    y = np.empty_like(y_ord)
    np.put_along_axis(y, order[..., None], y_ord, axis=1)
    return y.astype(np.float32), res.exec_time_ns


def kernel(**inputs):
    y, _ = _run(inputs, trace=False)
    return y
